# revision 27
# baseline (speedup 1.0000x reference)
# Trainium2 Bass kernel for PlaneFormer-style pairwise-MLP head model.
#
# Data parallel over batch B=64 -> 8 samples per NeuronCore.  Per sample the
# reference computes cat[i,j] = [emb_i, emb_j, g1, g2] (T,T,4D) followed by 4
# stacked MLP heads (1024->512->256->128->64->4) and masked reductions.
#
# Device pipeline (v2), everything feature-major (features on partitions):
#   gcat  = [g1;g2] chunks via tiny transposed matmuls (lhsT = X natural)
#   A''   = X@w0i + gcat@w0g + b0   (PSUM accumulation; gcat enters as a
#           stride-0-broadcast moving operand, b0 as a K=1 ones matmul)
#   B     = X@w0j  (head 3: all 32 j; heads 0-2: host-pregathered 16-col
#           window  j in [n0, n0+16) which provably contains all valid pairs)
#   h0    = relu(A''[:,i] + B[:,j])  -- DVE broadcast-AP add (bf16), then
#           in-place relu on the otherwise-idle GpSimd engine
#   L1-L3 matmul chain with relu+bias fused into PSUM->SBUF copies
#           (balanced between ACT and DVE by an est-cost counter)
#   L4 head 3 -> sigmoid (ACT) -> plane logits;  heads 0-2 -> DVE
#           scalar_tensor_tensor with the exact validity/pf weights.
# Host only reshapes inputs, builds the tiny mask tensors from num_planes,
# and applies b4 / final sigmoid on the (64,)-sized reduced outputs.

import numpy as np
import ml_dtypes

B, T, D = 64, 32, 256
H = 4
NCORES = 8
SPC = B // NCORES          # samples per core
WIN = 16
CW = WIN * WIN             # 256 window pair-columns (heads 0-2)
CF = T * T                 # 1024 full pair-columns (head 3)
F0, F1, F2, F3, F4 = 512, 256, 128, 64, 4
SG = 4                     # samples per processing group
NG = SPC // SG

BF16 = ml_dtypes.bfloat16

_PROG_CACHE = {}
LAST_RESULTS = None


def _build_program():
    import concourse.bass as bass
    import concourse.tile as tile
    from concourse import bacc, mybir
    from contextlib import ExitStack

    f32 = mybir.dt.float32
    bf = mybir.dt.bfloat16
    AF = mybir.ActivationFunctionType
    ALU = mybir.AluOpType

    nc = bacc.Bacc("TRN2", target_bir_lowering=False, debug=False,
                   num_devices=NCORES)

    xt_d = nc.dram_tensor("xt", [128, 2, SPC, T], bf, kind="ExternalInput").ap()
    xtw_d = nc.dram_tensor("xtw", [128, 2, SPC, WIN], bf, kind="ExternalInput").ap()
    xn_d = nc.dram_tensor("xnat", [T, SPC, D], bf, kind="ExternalInput").ap()
    mw_d = nc.dram_tensor("mwt", [T, SPC, 2], bf, kind="ExternalInput").ap()
    w0_d = nc.dram_tensor("w0s", [H, 128, 8, F0], bf, kind="ExternalInput").ap()
    w1_d = nc.dram_tensor("w1s", [H, 128, 4, 2, 128], bf, kind="ExternalInput").ap()
    w2_d = nc.dram_tensor("w2s", [128, H, 2, 128], bf, kind="ExternalInput").ap()
    w3_d = nc.dram_tensor("w3s", [128, H, F3], bf, kind="ExternalInput").ap()
    w4_d = nc.dram_tensor("w4s", [F3, H, F4], bf, kind="ExternalInput").ap()
    b0r_d = nc.dram_tensor("b0row", [1, H, 4, 128], bf, kind="ExternalInput").ap()
    bias_d = nc.dram_tensor("biases", [128, 33], f32, kind="ExternalInput").ap()
    wv_d = nc.dram_tensor("wv", [F4, SPC, CW], f32, kind="ExternalInput").ap()

    plane_d = nc.dram_tensor("plane", [F4, SPC, CF], f32, kind="ExternalOutput").ap()
    sums_d = nc.dram_tensor("sums", [F4, SPC * 3], f32, kind="ExternalOutput").ap()

    with tile.TileContext(nc) as tc:
        with ExitStack() as ctx:
            consts = ctx.enter_context(tc.tile_pool(name="consts", bufs=1))
            small = ctx.enter_context(tc.tile_pool(name="small", bufs=3))
            h0p = ctx.enter_context(tc.tile_pool(name="h0p", bufs=2))
            actp = ctx.enter_context(tc.tile_pool(name="acts", bufs=2))
            outp = ctx.enter_context(tc.tile_pool(name="outs", bufs=1))
            psb = ctx.enter_context(tc.tile_pool(name="psb", bufs=3, space="PSUM"))
            pss = ctx.enter_context(tc.tile_pool(name="pss", bufs=2, space="PSUM"))

            def load_const(ap_d, shape, dt, tag, eng=None):
                t = consts.tile(shape, dt, tag=tag)
                (eng or nc.sync).dma_start(out=t[:], in_=ap_d[:])
                return t

            xn = load_const(xn_d, [T, SPC, D], bf, "c_xn", nc.gpsimd)
            mw = load_const(mw_d, [T, SPC, 2], bf, "c_mw", nc.gpsimd)
            xt = load_const(xt_d, [128, 2, SPC, T], bf, "c_xt", nc.gpsimd)
            xtw = load_const(xtw_d, [128, 2, SPC, WIN], bf, "c_xtw", nc.gpsimd)
            bi = load_const(bias_d, [128, 33], f32, "c_bi", nc.gpsimd)
            # per-head weight DMAs, head 3 first so its pipeline starts early
            w0 = consts.tile([128, H, 8, F0], bf, tag="c_w0")
            w1 = consts.tile([128, H, 4, 2, 128], bf, tag="c_w1")
            w2 = consts.tile([128, H, 2, 128], bf, tag="c_w2")
            w3 = consts.tile([128, H, F3], bf, tag="c_w3")
            w4 = consts.tile([F3, H, F4], bf, tag="c_w4")
            for hh in (3, 0, 1, 2):
                nc.sync.dma_start(out=w0[:, hh, :, :], in_=w0_d[hh])
                nc.sync.dma_start(out=w1[:, hh, :, :, :], in_=w1_d[hh])
            nc.sync.dma_start(out=w2[:], in_=w2_d[:])
            nc.sync.dma_start(out=w3[:], in_=w3_d[:])
            nc.sync.dma_start(out=w4[:], in_=w4_d[:])
            wv = load_const(wv_d, [F4, SPC, CW], f32, "c_wv", nc.gpsimd)

            plane_sb = outp.tile([F4, SPC, CF], f32)
            sums_sb = outp.tile([F4, SPC * 3], f32)

            eng_cost = {"act": 0.0, "dve": 0.0}

            def fused_copy(out_ap, in_ap, bias_ap, fd, relu=True):
                c_act = (172 + fd) / 1.2
                c_dve = (120 + fd) / 0.96
                if eng_cost["act"] + c_act <= eng_cost["dve"] + c_dve:
                    eng_cost["act"] += c_act
                    nc.scalar.activation(
                        out=out_ap, in_=in_ap,
                        func=(AF.Relu if relu else
                              (AF.Copy if bias_ap is None else AF.Identity)),
                        bias=(bias_ap if bias_ap is not None else 0.0),
                        scale=1.0)
                else:
                    eng_cost["dve"] += c_dve
                    if relu:
                        nc.vector.tensor_scalar(
                            out=out_ap, in0=in_ap,
                            scalar1=bias_ap if bias_ap is not None else 0.0,
                            scalar2=0.0, op0=ALU.add, op1=ALU.max)
                    elif bias_ap is None:
                        nc.vector.tensor_copy(out_ap, in_ap)
                    else:
                        nc.vector.tensor_scalar_add(out_ap, in_ap, bias_ap)

            def mk_ap(base_ap, dims):
                return bass.AP(tensor=base_ap.tensor, offset=base_ap.offset,
                               ap=[base_ap.ap[0]] + dims)

            # ---- gcat: per sample, 4 chunks (g1 lo/hi, g2 lo/hi) ----------
            # gcps[:, s*4+c] = chunk c of [g1;g2] for sample s
            gcps = pss.tile([128, SPC * 4], mybir.dt.float32, tag="pa")
            # col layout: s*4 + dc*2 + q
            for s in range(SPC):
                for dc in range(2):
                    nc.tensor.matmul(
                        gcps[:, s * 4 + dc * 2:s * 4 + dc * 2 + 2],
                        lhsT=xn[:, s, dc * 128:(dc + 1) * 128],
                        rhs=mw[:, s, :], start=True, stop=True)
            gc = consts.tile([128, SPC * 4], bf, tag="c_gc")
            fused_copy(gc[:, :], gcps[:, :], None, SPC * 4, relu=False)
            # pre-broadcast gcat along i so AT's moving operand streams packed
            gcb = consts.tile([128, 4, SPC * T], bf, tag="c_gcb")
            for c in range(4):
                cc = (c % 2) * 2 + c // 2
                nc.vector.tensor_copy(
                    gcb[:, c, :],
                    mk_ap(gc[:, cc:cc + 1], [[4, SPC], [0, T]]))

            # ---- per-head A''/B for ALL samples (built lazily) ----
            ab_cache = {}

            def get_ab(h):
                if h in ab_cache:
                    return ab_cache[h]
                full = (h == 3)
                jn = T if full else WIN
                natA = SPC * T
                nbtA = SPC * jn
                btsb = small.tile([128, 4, nbtA], bf, tag=f"btsb{h}")
                xs = xt if full else xtw
                for mf in range(4):
                    bps = pss.tile([128, nbtA], mybir.dt.float32, tag="pa")
                    mfs = slice(mf * 128, (mf + 1) * 128)
                    for kd in range(2):
                        nc.tensor.matmul(
                            bps[:, :], lhsT=w0[:, h, 2 + kd, mfs],
                            rhs=xs[:, kd, :, :],
                            start=(kd == 0), stop=(kd == 1))
                    fused_copy(btsb[:, mf, :], bps[:, :], None, nbtA,
                               relu=False)
                absb = small.tile([128, 4, natA], bf, tag=f"absb{h}")
                for mf in range(4):
                    aps = pss.tile([128, natA], mybir.dt.float32, tag="pa")
                    mfs = slice(mf * 128, (mf + 1) * 128)
                    for kd in range(2):
                        nc.tensor.matmul(
                            aps[:, :], lhsT=w0[:, h, kd, mfs],
                            rhs=xt[:, kd, :, :],
                            start=(kd == 0), stop=False)
                    for c in range(4):
                        nc.tensor.matmul(
                            aps[:, :], lhsT=w0[:, h, 4 + c, mfs],
                            rhs=gcb[:, c, :], start=False, stop=(c == 3))
                    fused_copy(absb[:, mf, :], aps[:, :],
                               bi[:, h * 4 + mf:h * 4 + mf + 1], natA,
                               relu=False)
                ab_cache[h] = (absb, btsb)
                return ab_cache[h]

            # head 3 in 4 groups of 2 samples; heads 0-2 one group of 8.
            # cg = 2048 columns uniformly.
            tasks = [(3, 0), (3, 1), (0, 0), (3, 2), (1, 0), (3, 3),
                     (2, 0), (3, 4), (0, 1), (3, 5), (1, 1), (3, 6),
                     (2, 1), (3, 7)]
            for ti, (h, g) in enumerate(tasks):
                    full = (h == 3)
                    SGh = 1 if full else 4
                    s0 = g * SGh
                    cg = SGh * (CF if full else CW)  # group pair-columns
                    jn = T if full else WIN          # j per sample
                    absb, btsb = get_ab(h)

                    # ---- h0 = relu(A[:,s,i] + B[:,s,j]) ----
                    iN = T if full else WIN
                    h0sb = h0p.tile([128, 4, cg], bf, tag="h0")
                    for mf in range(4):
                        a_sl = absb[:, mf, s0 * T:]
                        in0 = mk_ap(a_sl, [[T, SGh], [1, iN], [0, jn]])
                        b_sl = btsb[:, mf, s0 * jn:]
                        in1 = mk_ap(b_sl, [[jn, SGh], [0, iN], [1, jn]])
                        out = h0sb[:, mf, :].rearrange(
                            "p (s i j) -> p s i j", s=SGh, i=iN)
                        eng_cost["dve"] += (151 + cg) / 0.96
                        nc.vector.tensor_tensor(out=out, in0=in0, in1=in1,
                                                op=ALU.add)
                    # in-place relu on DVE (single-src bf16 SBUF -> 4x mode)
                    eng_cost["dve"] += (58 + 4 * cg / 4) / 0.96
                    nc.vector.tensor_scalar_max(h0sb[:, :, :], h0sb[:, :, :],
                                                0.0)

                    # ---- L1: 512 -> 256 ----
                    h1sb = actp.tile([128, 2, cg], bf, tag="h1")
                    for mf in range(2):
                        for np_ in range(cg // 1024):
                            ps = psb.tile([128, 1024], mybir.dt.float32,
                                          tag="ps")
                            for hf in range(2):
                                c0 = np_ * 1024 + hf * 512
                                for kd in range(4):
                                    nc.tensor.matmul(
                                        ps[:, hf * 512:(hf + 1) * 512],
                                        lhsT=w1[:, h, kd, mf, :],
                                        rhs=h0sb[:, kd, c0:c0 + 512],
                                        start=(kd == 0), stop=(kd == 3))
                            fused_copy(
                                h1sb[:, mf, np_ * 1024:(np_ + 1) * 1024],
                                ps[:, :],
                                bi[:, 16 + h * 2 + mf:17 + h * 2 + mf], 1024)

                    # ---- L2: 256 -> 128 ----
                    h2sb = actp.tile([128, cg], bf, tag="h2")
                    for np_ in range(cg // 1024):
                        ps = psb.tile([128, 1024], mybir.dt.float32, tag="ps")
                        for hf in range(2):
                            c0 = np_ * 1024 + hf * 512
                            for kd in range(2):
                                nc.tensor.matmul(
                                    ps[:, hf * 512:(hf + 1) * 512],
                                    lhsT=w2[:, h, kd, :],
                                    rhs=h1sb[:, kd, c0:c0 + 512],
                                    start=(kd == 0), stop=(kd == 1))
                        fused_copy(h2sb[:, np_ * 1024:(np_ + 1) * 1024],
                                   ps[:, :], bi[:, 24 + h:25 + h], 1024)

                    # ---- L3: 128 -> 64 ----
                    h3sb = actp.tile([F3, cg], bf, tag="h3")
                    for np_ in range(cg // 1024):
                        ps = psb.tile([F3, 1024], mybir.dt.float32, tag="ps")
                        for hf in range(2):
                            c0 = np_ * 1024 + hf * 512
                            nc.tensor.matmul(
                                ps[:, hf * 512:(hf + 1) * 512],
                                lhsT=w3[:, h, :], rhs=h2sb[:, c0:c0 + 512],
                                start=True, stop=True)
                        fused_copy(h3sb[:, np_ * 1024:(np_ + 1) * 1024],
                                   ps[:, :], bi[0:F3, 28 + h:29 + h], 1024)

                    # ---- L4: 64 -> 4 + finals ----
                    for np_ in range(cg // 1024):
                        ps4 = psb.tile([F4, 1024], mybir.dt.float32, tag="ps")
                        for hf in range(2):
                            c0 = np_ * 1024 + hf * 512
                            nc.tensor.matmul(
                                ps4[:, hf * 512:(hf + 1) * 512],
                                lhsT=w4[:, h, :], rhs=h3sb[:, c0:c0 + 512],
                                start=True, stop=True)
                        if full:
                            # chunk np_ is exactly sample s0+np_'s plane
                            nc.scalar.activation(
                                out=plane_sb[:, s0 + np_, :], in_=ps4[:, :],
                                func=AF.Sigmoid, bias=bi[0:F4, 32:33],
                                scale=1.0)
                            nc.sync.dma_start(
                                out=plane_d[:, s0 + np_, :],
                                in_=plane_sb[:, s0 + np_, :])
                        else:
                            for sl in range(1024 // CW):
                                s = s0 + np_ * (1024 // CW) + sl
                                scr = small.tile([F4, CW],
                                                 mybir.dt.float32, tag="scr")
                                nc.vector.scalar_tensor_tensor(
                                    out=scr[:, :],
                                    in0=ps4[:, sl * CW:(sl + 1) * CW],
                                    scalar=1.0, in1=wv[:, s, :],
                                    op0=ALU.mult, op1=ALU.mult,
                                    accum_out=sums_sb[:, s * 3 + h:
                                                      s * 3 + h + 1])

            nc.sync.dma_start(out=sums_d[:], in_=sums_sb[:])

    nc.compile()
    return nc


def _host_prep(emb, num_planes, w0, b0, w1, b1, w2, b2, w3, b3, w4, b4):
    emb = np.asarray(emb, np.float32)
    npl = np.asarray(num_planes).astype(np.int64)
    n0 = npl[:, 0]
    n1 = npl[:, 1]
    assert n0.min() >= 1 and n1.min() >= 1 and n0.max() <= 16 and n1.max() <= 16

    idx = np.arange(T)
    m1 = idx[None, :] < n0[:, None]
    in2 = (idx[None, :] >= n0[:, None]) & (idx[None, :] < (n0 + n1)[:, None])
    mw1 = (m1 / n0[:, None]).astype(np.float32)
    mw2 = (in2 / n1[:, None]).astype(np.float32)

    # xt: [128, kd, s, i] = emb[s, i, kd*128+p]
    embT = emb.transpose(2, 0, 1)                       # (D, B, T)
    xt = np.ascontiguousarray(
        embT.reshape(2, 128, B, T).transpose(1, 0, 2, 3)).astype(BF16)
    # window-gathered columns j = n0[s] + jj
    xtw_f = np.zeros((2, 128, B, WIN), np.float32)
    embT_r = embT.reshape(2, 128, B, T)
    for b in range(B):
        xtw_f[:, :, b, :] = embT_r[:, :, b, n0[b]:n0[b] + WIN]
    xtw = np.ascontiguousarray(xtw_f.transpose(1, 0, 2, 3)).astype(BF16)

    xnat = np.ascontiguousarray(emb.transpose(1, 0, 2)).astype(BF16)  # (T,B,D)
    mwt = np.ascontiguousarray(
        np.stack([mw1, mw2], axis=-1).transpose(1, 0, 2)).astype(BF16)

    w0s = np.ascontiguousarray(
        np.asarray(w0, np.float32).reshape(H, 8, 128, F0).transpose(0, 2, 1, 3)
    ).astype(BF16)
    w1s = np.ascontiguousarray(
        np.asarray(w1, np.float32).reshape(H, 4, 128, 2, 128)
        .transpose(0, 2, 1, 3, 4)).astype(BF16)
    w2s = np.ascontiguousarray(
        np.asarray(w2, np.float32).reshape(H, 2, 128, 128).transpose(2, 0, 1, 3)
    ).astype(BF16)
    w3s = np.ascontiguousarray(
        np.asarray(w3, np.float32).transpose(1, 0, 2)).astype(BF16)
    w4s = np.ascontiguousarray(
        np.asarray(w4, np.float32).transpose(1, 0, 2)).astype(BF16)
    b0row = np.ascontiguousarray(
        np.asarray(b0, np.float32).reshape(1, H, 4, 128)).astype(BF16)

    biases = np.zeros((128, 33), np.float32)
    biases[:, 0:16] = np.asarray(b0, np.float32).reshape(H, 4, 128) \
        .transpose(2, 0, 1).reshape(128, 16)
    biases[:, 16:24] = np.asarray(b1, np.float32).reshape(H, 2, 128) \
        .transpose(2, 0, 1).reshape(128, 8)
    biases[:, 24:28] = np.asarray(b2, np.float32).T
    biases[0:F3, 28:32] = np.asarray(b3, np.float32).T
    biases[0:F4, 32] = np.asarray(b4, np.float32)[3]

    cw = np.arange(CW)
    iw, jw = cw // WIN, cw % WIN
    pf = (n0 * n1).astype(np.float32)
    wvw = ((iw[None, :] < n0[:, None]) & (jw[None, :] < n1[:, None])) \
        / pf[:, None]
    wvw = np.broadcast_to(wvw[:, None, :].astype(np.float32), (B, F4, CW))

    in_maps = []
    for c in range(NCORES):
        sl = slice(c * SPC, (c + 1) * SPC)
        in_maps.append({
            "xt": np.ascontiguousarray(xt[:, :, sl, :]),
            "xtw": np.ascontiguousarray(xtw[:, :, sl, :]),
            "xnat": np.ascontiguousarray(xnat[:, sl, :]),
            "mwt": np.ascontiguousarray(mwt[:, sl, :]),
            "w0s": w0s, "w1s": w1s, "w2s": w2s, "w3s": w3s, "w4s": w4s,
            "b0row": b0row, "biases": biases,
            "wv": np.ascontiguousarray(wvw[sl].transpose(1, 0, 2)),
        })
    meta = dict(m1=m1, in2=in2, b4=np.asarray(b4, np.float32))
    return in_maps, meta


def _host_post(results, meta):
    b4 = meta["b4"]
    cam = np.zeros(B, np.float32)
    rot = np.zeros((B, 4), np.float32)
    trn = np.zeros((B, 3), np.float32)
    plane = np.zeros((B, T, T), np.float32)
    for c in range(NCORES):
        pl = results[c]["plane"]      # (4, SPC, CF)
        sm = results[c]["sums"]       # (4, SPC*3)
        for s in range(SPC):
            b = c * SPC + s
            plane[b] = pl[0, s].reshape(T, T)
            cam[b] = sm[0, s * 3 + 0] + b4[0, 0]
            rot[b] = sm[:, s * 3 + 1] + b4[1]
            trn[b] = sm[0:3, s * 3 + 2] + b4[2, :3]
    cam = (1.0 / (1.0 + np.exp(-cam.astype(np.float64)))).astype(np.float32)
    valid = meta["m1"][:, :, None] & meta["in2"][:, None, :]
    return cam, rot, trn, plane, valid


def kernel(emb, num_planes, w0, b0, w1, b1, w2, b2, w3, b3, w4, b4,
           _trace=False):
    global LAST_RESULTS
    from concourse.bass_utils import run_bass_kernel_spmd

    if "prog" not in _PROG_CACHE:
        _PROG_CACHE["prog"] = _build_program()
    nc = _PROG_CACHE["prog"]

    in_maps, meta = _host_prep(emb, num_planes, w0, b0, w1, b1,
                               w2, b2, w3, b3, w4, b4)
    res = run_bass_kernel_spmd(nc, in_maps, list(range(NCORES)),
                               trace=_trace)
    LAST_RESULTS = res
    return _host_post(res.results, meta)


# revision 28
# speedup vs baseline: 1.1598x; 1.1598x over previous
# Trainium2 Bass kernel for PlaneFormer-style pairwise-MLP head model.
#
# Data parallel over batch B=64 -> 8 samples per NeuronCore.  Per sample the
# reference computes cat[i,j] = [emb_i, emb_j, g1, g2] (T,T,4D) followed by 4
# stacked MLP heads (1024->512->256->128->64->4) and masked reductions.
#
# Device pipeline (v2), everything feature-major (features on partitions):
#   gcat  = [g1;g2] chunks via tiny transposed matmuls (lhsT = X natural)
#   A''   = X@w0i + gcat@w0g + b0   (PSUM accumulation; gcat enters as a
#           stride-0-broadcast moving operand, b0 as a K=1 ones matmul)
#   B     = X@w0j  (head 3: all 32 j; heads 0-2: host-pregathered 16-col
#           window  j in [n0, n0+16) which provably contains all valid pairs)
#   h0    = relu(A''[:,i] + B[:,j])  -- DVE broadcast-AP add (bf16), then
#           in-place relu on the otherwise-idle GpSimd engine
#   L1-L3 matmul chain with relu+bias fused into PSUM->SBUF copies
#           (balanced between ACT and DVE by an est-cost counter)
#   L4 head 3 -> sigmoid (ACT) -> plane logits;  heads 0-2 -> DVE
#           scalar_tensor_tensor with the exact validity/pf weights.
# Host only reshapes inputs, builds the tiny mask tensors from num_planes,
# and applies b4 / final sigmoid on the (64,)-sized reduced outputs.

import numpy as np
import ml_dtypes

B, T, D = 64, 32, 256
H = 4
NCORES = 8
SPC = B // NCORES          # samples per core
WIN = 16
CW = WIN * WIN             # 256 window pair-columns (heads 0-2)
CF = T * T                 # 1024 full pair-columns (head 3)
F0, F1, F2, F3, F4 = 512, 256, 128, 64, 4
SG = 4                     # samples per processing group
NG = SPC // SG

BF16 = ml_dtypes.bfloat16

_PROG_CACHE = {}
LAST_RESULTS = None


def _build_program():
    import concourse.bass as bass
    import concourse.tile as tile
    from concourse import bacc, mybir
    from contextlib import ExitStack

    f32 = mybir.dt.float32
    bf = mybir.dt.bfloat16
    AF = mybir.ActivationFunctionType
    ALU = mybir.AluOpType

    nc = bacc.Bacc("TRN2", target_bir_lowering=False, debug=False,
                   num_devices=NCORES)

    xt_d = nc.dram_tensor("xt", [128, 2, SPC, T], bf, kind="ExternalInput").ap()
    xtw_d = nc.dram_tensor("xtw", [128, 2, SPC, WIN], bf, kind="ExternalInput").ap()
    xn_d = nc.dram_tensor("xnat", [T, SPC, D], bf, kind="ExternalInput").ap()
    mw_d = nc.dram_tensor("mwt", [T, SPC, 2], bf, kind="ExternalInput").ap()
    w0_d = nc.dram_tensor("w0s", [H, 128, 8, F0], bf, kind="ExternalInput").ap()
    w1_d = nc.dram_tensor("w1s", [H, 128, 4, 2, 128], bf, kind="ExternalInput").ap()
    w2_d = nc.dram_tensor("w2s", [128, H, 2, 128], bf, kind="ExternalInput").ap()
    w3_d = nc.dram_tensor("w3s", [128, H, F3], bf, kind="ExternalInput").ap()
    w4_d = nc.dram_tensor("w4s", [F3, H, F4], bf, kind="ExternalInput").ap()
    b0r_d = nc.dram_tensor("b0row", [1, H, 4, 128], bf, kind="ExternalInput").ap()
    bias_d = nc.dram_tensor("biases", [128, 33], f32, kind="ExternalInput").ap()
    wv_d = nc.dram_tensor("wv", [F4, SPC, CW], f32, kind="ExternalInput").ap()

    plane_d = nc.dram_tensor("plane", [F4, SPC, CF], f32, kind="ExternalOutput").ap()
    sums_d = nc.dram_tensor("sums", [F4, SPC * 3], f32, kind="ExternalOutput").ap()

    with tile.TileContext(nc) as tc:
        with ExitStack() as ctx:
            consts = ctx.enter_context(tc.tile_pool(name="consts", bufs=1))
            small = ctx.enter_context(tc.tile_pool(name="small", bufs=3))
            h0p = ctx.enter_context(tc.tile_pool(name="h0p", bufs=2))
            actp = ctx.enter_context(tc.tile_pool(name="acts", bufs=2))
            outp = ctx.enter_context(tc.tile_pool(name="outs", bufs=1))
            psb = ctx.enter_context(tc.tile_pool(name="psb", bufs=3, space="PSUM"))
            pss = ctx.enter_context(tc.tile_pool(name="pss", bufs=2, space="PSUM"))

            def load_const(ap_d, shape, dt, tag, eng=None):
                t = consts.tile(shape, dt, tag=tag)
                (eng or nc.sync).dma_start(out=t[:], in_=ap_d[:])
                return t

            xn = load_const(xn_d, [T, SPC, D], bf, "c_xn", nc.gpsimd)
            mw = load_const(mw_d, [T, SPC, 2], bf, "c_mw", nc.gpsimd)
            xt = load_const(xt_d, [128, 2, SPC, T], bf, "c_xt", nc.gpsimd)
            xtw = load_const(xtw_d, [128, 2, SPC, WIN], bf, "c_xtw", nc.gpsimd)
            bi = load_const(bias_d, [128, 33], f32, "c_bi", nc.gpsimd)
            # per-head weight DMAs, head 3 first so its pipeline starts early
            w0 = consts.tile([128, H, 8, F0], bf, tag="c_w0")
            w1 = consts.tile([128, H, 4, 2, 128], bf, tag="c_w1")
            w2 = consts.tile([128, H, 2, 128], bf, tag="c_w2")
            w3 = consts.tile([128, H, F3], bf, tag="c_w3")
            w4 = consts.tile([F3, H, F4], bf, tag="c_w4")
            for hh in (3, 0, 1, 2):
                nc.sync.dma_start(out=w0[:, hh, :, :], in_=w0_d[hh])
                nc.sync.dma_start(out=w1[:, hh, :, :, :], in_=w1_d[hh])
            nc.sync.dma_start(out=w2[:], in_=w2_d[:])
            nc.sync.dma_start(out=w3[:], in_=w3_d[:])
            nc.sync.dma_start(out=w4[:], in_=w4_d[:])
            wv = load_const(wv_d, [F4, SPC, CW], f32, "c_wv", nc.gpsimd)

            plane_sb = outp.tile([F4, SPC, CF], f32)
            sums_sb = outp.tile([F4, SPC * 3], f32)

            eng_cost = {"act": 0.0, "dve": 0.0}

            def fused_copy(out_ap, in_ap, bias_ap, fd, relu=True):
                c_act = (172 + fd) / 1.2
                c_dve = (120 + fd) / 0.96
                if eng_cost["act"] + c_act <= eng_cost["dve"] + c_dve:
                    eng_cost["act"] += c_act
                    nc.scalar.activation(
                        out=out_ap, in_=in_ap,
                        func=(AF.Relu if relu else
                              (AF.Copy if bias_ap is None else AF.Identity)),
                        bias=(bias_ap if bias_ap is not None else 0.0),
                        scale=1.0)
                else:
                    eng_cost["dve"] += c_dve
                    if relu:
                        nc.vector.tensor_scalar(
                            out=out_ap, in0=in_ap,
                            scalar1=bias_ap if bias_ap is not None else 0.0,
                            scalar2=0.0, op0=ALU.add, op1=ALU.max)
                    elif bias_ap is None:
                        nc.vector.tensor_copy(out_ap, in_ap)
                    else:
                        nc.vector.tensor_scalar_add(out_ap, in_ap, bias_ap)

            def mk_ap(base_ap, dims):
                return bass.AP(tensor=base_ap.tensor, offset=base_ap.offset,
                               ap=[base_ap.ap[0]] + dims)

            # ---- gcat: per sample, 4 chunks (g1 lo/hi, g2 lo/hi) ----------
            # gcps[:, s*4+c] = chunk c of [g1;g2] for sample s
            gcps = pss.tile([128, SPC * 4], mybir.dt.float32, tag="pa")
            # col layout: s*4 + dc*2 + q
            for s in range(SPC):
                for dc in range(2):
                    nc.tensor.matmul(
                        gcps[:, s * 4 + dc * 2:s * 4 + dc * 2 + 2],
                        lhsT=xn[:, s, dc * 128:(dc + 1) * 128],
                        rhs=mw[:, s, :], start=True, stop=True)
            gc = consts.tile([128, SPC * 4], bf, tag="c_gc")
            fused_copy(gc[:, :], gcps[:, :], None, SPC * 4, relu=False)
            # pre-broadcast gcat along i so AT's moving operand streams packed
            gcb = consts.tile([128, 4, SPC * T], bf, tag="c_gcb")
            for c in range(4):
                cc = (c % 2) * 2 + c // 2
                nc.vector.tensor_copy(
                    gcb[:, c, :],
                    mk_ap(gc[:, cc:cc + 1], [[4, SPC], [0, T]]))

            # ---- per-head A''/B for ALL samples (built lazily) ----
            ab_cache = {}

            def get_ab(h):
                if h in ab_cache:
                    return ab_cache[h]
                full = (h == 3)
                jn = T if full else WIN
                natA = SPC * T
                nbtA = SPC * jn
                btsb = small.tile([128, 4, nbtA], bf, tag=f"btsb{h}")
                xs = xt if full else xtw
                for mf in range(4):
                    bps = pss.tile([128, nbtA], mybir.dt.float32, tag="pa")
                    mfs = slice(mf * 128, (mf + 1) * 128)
                    for kd in range(2):
                        nc.tensor.matmul(
                            bps[:, :], lhsT=w0[:, h, 2 + kd, mfs],
                            rhs=xs[:, kd, :, :],
                            start=(kd == 0), stop=(kd == 1))
                    fused_copy(btsb[:, mf, :], bps[:, :], None, nbtA,
                               relu=False)
                absb = small.tile([128, 4, natA], bf, tag=f"absb{h}")
                for mf in range(4):
                    aps = pss.tile([128, natA], mybir.dt.float32, tag="pa")
                    mfs = slice(mf * 128, (mf + 1) * 128)
                    for kd in range(2):
                        nc.tensor.matmul(
                            aps[:, :], lhsT=w0[:, h, kd, mfs],
                            rhs=xt[:, kd, :, :],
                            start=(kd == 0), stop=False)
                    for c in range(4):
                        nc.tensor.matmul(
                            aps[:, :], lhsT=w0[:, h, 4 + c, mfs],
                            rhs=gcb[:, c, :], start=False, stop=(c == 3))
                    fused_copy(absb[:, mf, :], aps[:, :],
                               bi[:, h * 4 + mf:h * 4 + mf + 1], natA,
                               relu=False)
                ab_cache[h] = (absb, btsb)
                return ab_cache[h]

            # head 3 in 4 groups of 2 samples; heads 0-2 one group of 8.
            # cg = 2048 columns uniformly.
            tasks = [(3, 0), (0, 0), (3, 1), (1, 0), (3, 2), (2, 0), (3, 3)]
            for ti, (h, g) in enumerate(tasks):
                    full = (h == 3)
                    SGh = 2 if full else SPC
                    s0 = g * SGh
                    cg = SGh * (CF if full else CW)  # group pair-columns
                    jn = T if full else WIN          # j per sample
                    absb, btsb = get_ab(h)

                    # ---- h0 = relu(A[:,s,i] + B[:,s,j]) ----
                    iN = T if full else WIN
                    h0sb = h0p.tile([128, 4, cg], bf, tag="h0")
                    for mf in range(4):
                        a_sl = absb[:, mf, s0 * T:]
                        in0 = mk_ap(a_sl, [[T, SGh], [1, iN], [0, jn]])
                        b_sl = btsb[:, mf, s0 * jn:]
                        in1 = mk_ap(b_sl, [[jn, SGh], [0, iN], [1, jn]])
                        out = h0sb[:, mf, :].rearrange(
                            "p (s i j) -> p s i j", s=SGh, i=iN)
                        eng_cost["dve"] += (151 + cg) / 0.96
                        nc.vector.tensor_tensor(out=out, in0=in0, in1=in1,
                                                op=ALU.add)
                    # in-place relu on DVE (single-src bf16 SBUF -> 4x mode)
                    eng_cost["dve"] += (58 + 4 * cg / 4) / 0.96
                    nc.vector.tensor_scalar_max(h0sb[:, :, :], h0sb[:, :, :],
                                                0.0)

                    # ---- L1: 512 -> 256 ----
                    h1sb = actp.tile([128, 2, cg], bf, tag="h1")
                    for mf in range(2):
                        for np_ in range(cg // 1024):
                            ps = psb.tile([128, 1024], mybir.dt.float32,
                                          tag="ps")
                            for hf in range(2):
                                c0 = np_ * 1024 + hf * 512
                                for kd in range(4):
                                    nc.tensor.matmul(
                                        ps[:, hf * 512:(hf + 1) * 512],
                                        lhsT=w1[:, h, kd, mf, :],
                                        rhs=h0sb[:, kd, c0:c0 + 512],
                                        start=(kd == 0), stop=(kd == 3))
                            fused_copy(
                                h1sb[:, mf, np_ * 1024:(np_ + 1) * 1024],
                                ps[:, :],
                                bi[:, 16 + h * 2 + mf:17 + h * 2 + mf], 1024)

                    # ---- L2: 256 -> 128 ----
                    h2sb = actp.tile([128, cg], bf, tag="h2")
                    for np_ in range(cg // 1024):
                        ps = psb.tile([128, 1024], mybir.dt.float32, tag="ps")
                        for hf in range(2):
                            c0 = np_ * 1024 + hf * 512
                            for kd in range(2):
                                nc.tensor.matmul(
                                    ps[:, hf * 512:(hf + 1) * 512],
                                    lhsT=w2[:, h, kd, :],
                                    rhs=h1sb[:, kd, c0:c0 + 512],
                                    start=(kd == 0), stop=(kd == 1))
                        fused_copy(h2sb[:, np_ * 1024:(np_ + 1) * 1024],
                                   ps[:, :], bi[:, 24 + h:25 + h], 1024)

                    # ---- L3: 128 -> 64 ----
                    h3sb = actp.tile([F3, cg], bf, tag="h3")
                    for np_ in range(cg // 1024):
                        ps = psb.tile([F3, 1024], mybir.dt.float32, tag="ps")
                        for hf in range(2):
                            c0 = np_ * 1024 + hf * 512
                            nc.tensor.matmul(
                                ps[:, hf * 512:(hf + 1) * 512],
                                lhsT=w3[:, h, :], rhs=h2sb[:, c0:c0 + 512],
                                start=True, stop=True)
                        fused_copy(h3sb[:, np_ * 1024:(np_ + 1) * 1024],
                                   ps[:, :], bi[0:F3, 28 + h:29 + h], 1024)

                    # ---- L4: 64 -> 4 + finals ----
                    for np_ in range(cg // 1024):
                        ps4 = psb.tile([F4, 1024], mybir.dt.float32, tag="ps")
                        for hf in range(2):
                            c0 = np_ * 1024 + hf * 512
                            nc.tensor.matmul(
                                ps4[:, hf * 512:(hf + 1) * 512],
                                lhsT=w4[:, h, :], rhs=h3sb[:, c0:c0 + 512],
                                start=True, stop=True)
                        if full:
                            # chunk np_ is exactly sample s0+np_'s plane
                            nc.scalar.activation(
                                out=plane_sb[:, s0 + np_, :], in_=ps4[:, :],
                                func=AF.Sigmoid, bias=bi[0:F4, 32:33],
                                scale=1.0)
                            nc.sync.dma_start(
                                out=plane_d[:, s0 + np_, :],
                                in_=plane_sb[:, s0 + np_, :])
                        else:
                            for sl in range(1024 // CW):
                                s = s0 + np_ * (1024 // CW) + sl
                                scr = small.tile([F4, CW],
                                                 mybir.dt.float32, tag="scr")
                                nc.vector.scalar_tensor_tensor(
                                    out=scr[:, :],
                                    in0=ps4[:, sl * CW:(sl + 1) * CW],
                                    scalar=1.0, in1=wv[:, s, :],
                                    op0=ALU.mult, op1=ALU.mult,
                                    accum_out=sums_sb[:, s * 3 + h:
                                                      s * 3 + h + 1])

            nc.sync.dma_start(out=sums_d[:], in_=sums_sb[:])

    nc.compile()
    return nc


def _host_prep(emb, num_planes, w0, b0, w1, b1, w2, b2, w3, b3, w4, b4):
    emb = np.asarray(emb, np.float32)
    npl = np.asarray(num_planes).astype(np.int64)
    n0 = npl[:, 0]
    n1 = npl[:, 1]
    assert n0.min() >= 1 and n1.min() >= 1 and n0.max() <= 16 and n1.max() <= 16

    idx = np.arange(T)
    m1 = idx[None, :] < n0[:, None]
    in2 = (idx[None, :] >= n0[:, None]) & (idx[None, :] < (n0 + n1)[:, None])
    mw1 = (m1 / n0[:, None]).astype(np.float32)
    mw2 = (in2 / n1[:, None]).astype(np.float32)

    # xt: [128, kd, s, i] = emb[s, i, kd*128+p]
    embT = emb.transpose(2, 0, 1)                       # (D, B, T)
    xt = np.ascontiguousarray(
        embT.reshape(2, 128, B, T).transpose(1, 0, 2, 3)).astype(BF16)
    # window-gathered columns j = n0[s] + jj
    xtw_f = np.zeros((2, 128, B, WIN), np.float32)
    embT_r = embT.reshape(2, 128, B, T)
    for b in range(B):
        xtw_f[:, :, b, :] = embT_r[:, :, b, n0[b]:n0[b] + WIN]
    xtw = np.ascontiguousarray(xtw_f.transpose(1, 0, 2, 3)).astype(BF16)

    xnat = np.ascontiguousarray(emb.transpose(1, 0, 2)).astype(BF16)  # (T,B,D)
    mwt = np.ascontiguousarray(
        np.stack([mw1, mw2], axis=-1).transpose(1, 0, 2)).astype(BF16)

    w0s = np.ascontiguousarray(
        np.asarray(w0, np.float32).reshape(H, 8, 128, F0).transpose(0, 2, 1, 3)
    ).astype(BF16)
    w1s = np.ascontiguousarray(
        np.asarray(w1, np.float32).reshape(H, 4, 128, 2, 128)
        .transpose(0, 2, 1, 3, 4)).astype(BF16)
    w2s = np.ascontiguousarray(
        np.asarray(w2, np.float32).reshape(H, 2, 128, 128).transpose(2, 0, 1, 3)
    ).astype(BF16)
    w3s = np.ascontiguousarray(
        np.asarray(w3, np.float32).transpose(1, 0, 2)).astype(BF16)
    w4s = np.ascontiguousarray(
        np.asarray(w4, np.float32).transpose(1, 0, 2)).astype(BF16)
    b0row = np.ascontiguousarray(
        np.asarray(b0, np.float32).reshape(1, H, 4, 128)).astype(BF16)

    biases = np.zeros((128, 33), np.float32)
    biases[:, 0:16] = np.asarray(b0, np.float32).reshape(H, 4, 128) \
        .transpose(2, 0, 1).reshape(128, 16)
    biases[:, 16:24] = np.asarray(b1, np.float32).reshape(H, 2, 128) \
        .transpose(2, 0, 1).reshape(128, 8)
    biases[:, 24:28] = np.asarray(b2, np.float32).T
    biases[0:F3, 28:32] = np.asarray(b3, np.float32).T
    biases[0:F4, 32] = np.asarray(b4, np.float32)[3]

    cw = np.arange(CW)
    iw, jw = cw // WIN, cw % WIN
    pf = (n0 * n1).astype(np.float32)
    wvw = ((iw[None, :] < n0[:, None]) & (jw[None, :] < n1[:, None])) \
        / pf[:, None]
    wvw = np.broadcast_to(wvw[:, None, :].astype(np.float32), (B, F4, CW))

    in_maps = []
    for c in range(NCORES):
        sl = slice(c * SPC, (c + 1) * SPC)
        in_maps.append({
            "xt": np.ascontiguousarray(xt[:, :, sl, :]),
            "xtw": np.ascontiguousarray(xtw[:, :, sl, :]),
            "xnat": np.ascontiguousarray(xnat[:, sl, :]),
            "mwt": np.ascontiguousarray(mwt[:, sl, :]),
            "w0s": w0s, "w1s": w1s, "w2s": w2s, "w3s": w3s, "w4s": w4s,
            "b0row": b0row, "biases": biases,
            "wv": np.ascontiguousarray(wvw[sl].transpose(1, 0, 2)),
        })
    meta = dict(m1=m1, in2=in2, b4=np.asarray(b4, np.float32))
    return in_maps, meta


def _host_post(results, meta):
    b4 = meta["b4"]
    cam = np.zeros(B, np.float32)
    rot = np.zeros((B, 4), np.float32)
    trn = np.zeros((B, 3), np.float32)
    plane = np.zeros((B, T, T), np.float32)
    for c in range(NCORES):
        pl = results[c]["plane"]      # (4, SPC, CF)
        sm = results[c]["sums"]       # (4, SPC*3)
        for s in range(SPC):
            b = c * SPC + s
            plane[b] = pl[0, s].reshape(T, T)
            cam[b] = sm[0, s * 3 + 0] + b4[0, 0]
            rot[b] = sm[:, s * 3 + 1] + b4[1]
            trn[b] = sm[0:3, s * 3 + 2] + b4[2, :3]
    cam = (1.0 / (1.0 + np.exp(-cam.astype(np.float64)))).astype(np.float32)
    valid = meta["m1"][:, :, None] & meta["in2"][:, None, :]
    return cam, rot, trn, plane, valid


def kernel(emb, num_planes, w0, b0, w1, b1, w2, b2, w3, b3, w4, b4,
           _trace=False):
    global LAST_RESULTS
    from concourse.bass_utils import run_bass_kernel_spmd

    if "prog" not in _PROG_CACHE:
        _PROG_CACHE["prog"] = _build_program()
    nc = _PROG_CACHE["prog"]

    in_maps, meta = _host_prep(emb, num_planes, w0, b0, w1, b1,
                               w2, b2, w3, b3, w4, b4)
    res = run_bass_kernel_spmd(nc, in_maps, list(range(NCORES)),
                               trace=_trace)
    LAST_RESULTS = res
    return _host_post(res.results, meta)


# revision 29
# speedup vs baseline: 1.4029x; 1.2096x over previous
# Trainium2 Bass kernel for PlaneFormer-style pairwise-MLP head model.
#
# Data parallel over batch B=64 -> 8 samples per NeuronCore.  Per sample the
# reference computes cat[i,j] = [emb_i, emb_j, g1, g2] (T,T,4D) followed by 4
# stacked MLP heads (1024->512->256->128->64->4) and masked reductions.
#
# Device pipeline (v2), everything feature-major (features on partitions):
#   gcat  = [g1;g2] chunks via tiny transposed matmuls (lhsT = X natural)
#   A''   = X@w0i + gcat@w0g + b0   (PSUM accumulation; gcat enters as a
#           stride-0-broadcast moving operand, b0 as a K=1 ones matmul)
#   B     = X@w0j  (head 3: all 32 j; heads 0-2: host-pregathered 16-col
#           window  j in [n0, n0+16) which provably contains all valid pairs)
#   h0    = relu(A''[:,i] + B[:,j])  -- DVE broadcast-AP add (bf16), then
#           in-place relu on the otherwise-idle GpSimd engine
#   L1-L3 matmul chain with relu+bias fused into PSUM->SBUF copies
#           (balanced between ACT and DVE by an est-cost counter)
#   L4 head 3 -> sigmoid (ACT) -> plane logits;  heads 0-2 -> DVE
#           scalar_tensor_tensor with the exact validity/pf weights.
# Host only reshapes inputs, builds the tiny mask tensors from num_planes,
# and applies b4 / final sigmoid on the (64,)-sized reduced outputs.

import numpy as np
import ml_dtypes

B, T, D = 64, 32, 256
H = 4
NCORES = 8
SPC = B // NCORES          # samples per core
WIN = 16
CW = WIN * WIN             # 256 window pair-columns (heads 0-2)
CF = T * T                 # 1024 full pair-columns (head 3)
F0, F1, F2, F3, F4 = 512, 256, 128, 64, 4
SG = 4                     # samples per processing group
NG = SPC // SG

BF16 = ml_dtypes.bfloat16

_PROG_CACHE = {}
LAST_RESULTS = None


def _build_program():
    import concourse.bass as bass
    import concourse.tile as tile
    from concourse import bacc, mybir
    from contextlib import ExitStack

    f32 = mybir.dt.float32
    bf = mybir.dt.bfloat16
    AF = mybir.ActivationFunctionType
    ALU = mybir.AluOpType

    nc = bacc.Bacc("TRN2", target_bir_lowering=False, debug=False,
                   num_devices=NCORES)

    xt_d = nc.dram_tensor("xt", [128, 2, SPC, T], bf, kind="ExternalInput").ap()
    xtw_d = nc.dram_tensor("xtw", [128, 2, SPC, WIN], bf, kind="ExternalInput").ap()
    xn_d = nc.dram_tensor("xnat", [T, SPC, D], bf, kind="ExternalInput").ap()
    mw_d = nc.dram_tensor("mwt", [T, SPC, 2], bf, kind="ExternalInput").ap()
    w0_d = nc.dram_tensor("w0s", [H, 128, 8, F0], bf, kind="ExternalInput").ap()
    w1_d = nc.dram_tensor("w1s", [H, 128, 4, 2, 128], bf, kind="ExternalInput").ap()
    w2_d = nc.dram_tensor("w2s", [128, H, 2, 128], bf, kind="ExternalInput").ap()
    w3_d = nc.dram_tensor("w3s", [128, H, F3], bf, kind="ExternalInput").ap()
    w4_d = nc.dram_tensor("w4s", [F3, H, F4], bf, kind="ExternalInput").ap()
    b0r_d = nc.dram_tensor("b0row", [1, H, 4, 128], bf, kind="ExternalInput").ap()
    bias_d = nc.dram_tensor("biases", [128, 33], f32, kind="ExternalInput").ap()
    wv_d = nc.dram_tensor("wv", [F4, SPC, CW], f32, kind="ExternalInput").ap()

    plane_d = nc.dram_tensor("plane", [F4, SPC, CF], f32, kind="ExternalOutput").ap()
    sums_d = nc.dram_tensor("sums", [F4, SPC * 3], f32, kind="ExternalOutput").ap()

    with tile.TileContext(nc) as tc:
        with ExitStack() as ctx:
            consts = ctx.enter_context(tc.tile_pool(name="consts", bufs=1))
            small = ctx.enter_context(tc.tile_pool(name="small", bufs=3))
            h0p = ctx.enter_context(tc.tile_pool(name="h0p", bufs=2))
            actp = ctx.enter_context(tc.tile_pool(name="acts", bufs=2))
            outp = ctx.enter_context(tc.tile_pool(name="outs", bufs=1))
            psb = ctx.enter_context(tc.tile_pool(name="psb", bufs=3, space="PSUM"))
            pss = ctx.enter_context(tc.tile_pool(name="pss", bufs=2, space="PSUM"))

            def load_const(ap_d, shape, dt, tag, eng=None):
                t = consts.tile(shape, dt, tag=tag)
                (eng or nc.sync).dma_start(out=t[:], in_=ap_d[:])
                return t

            xn = load_const(xn_d, [T, SPC, D], bf, "c_xn", nc.gpsimd)
            mw = load_const(mw_d, [T, SPC, 2], bf, "c_mw", nc.gpsimd)
            xt = load_const(xt_d, [128, 2, SPC, T], bf, "c_xt", nc.gpsimd)
            xtw = load_const(xtw_d, [128, 2, SPC, WIN], bf, "c_xtw", nc.gpsimd)
            bi = load_const(bias_d, [128, 33], f32, "c_bi", nc.gpsimd)
            # per-head weight DMAs, head 3 first so its pipeline starts early
            w0 = consts.tile([128, H, 8, F0], bf, tag="c_w0")
            w1 = consts.tile([128, H, 4, 2, 128], bf, tag="c_w1")
            w2 = consts.tile([128, H, 2, 128], bf, tag="c_w2")
            w3 = consts.tile([128, H, F3], bf, tag="c_w3")
            w4 = consts.tile([F3, H, F4], bf, tag="c_w4")
            for hh in (3, 0, 1, 2):
                nc.sync.dma_start(out=w0[:, hh, :, :], in_=w0_d[hh])
                nc.sync.dma_start(out=w1[:, hh, :, :, :], in_=w1_d[hh])
            nc.sync.dma_start(out=w2[:], in_=w2_d[:])
            nc.sync.dma_start(out=w3[:], in_=w3_d[:])
            nc.sync.dma_start(out=w4[:], in_=w4_d[:])
            wv = load_const(wv_d, [F4, SPC, CW], f32, "c_wv", nc.gpsimd)

            plane_sb = outp.tile([F4, SPC, CF], f32)
            sums_sb = outp.tile([F4, SPC * 3], f32)

            eng_cost = {"act": 0.0, "dve": 0.0}

            def fused_copy(out_ap, in_ap, bias_ap, fd, relu=True):
                c_act = (172 + fd) / 1.2
                c_dve = (120 + fd) / 0.96
                if eng_cost["act"] + c_act <= eng_cost["dve"] + c_dve:
                    eng_cost["act"] += c_act
                    nc.scalar.activation(
                        out=out_ap, in_=in_ap,
                        func=(AF.Relu if relu else
                              (AF.Copy if bias_ap is None else AF.Identity)),
                        bias=(bias_ap if bias_ap is not None else 0.0),
                        scale=1.0)
                else:
                    eng_cost["dve"] += c_dve
                    if relu:
                        nc.vector.tensor_scalar(
                            out=out_ap, in0=in_ap,
                            scalar1=bias_ap if bias_ap is not None else 0.0,
                            scalar2=0.0, op0=ALU.add, op1=ALU.max)
                    elif bias_ap is None:
                        nc.vector.tensor_copy(out_ap, in_ap)
                    else:
                        nc.vector.tensor_scalar_add(out_ap, in_ap, bias_ap)

            def mk_ap(base_ap, dims):
                return bass.AP(tensor=base_ap.tensor, offset=base_ap.offset,
                               ap=[base_ap.ap[0]] + dims)

            # ---- gcat: per sample, 4 chunks (g1 lo/hi, g2 lo/hi) ----------
            # gcps[:, s*4+c] = chunk c of [g1;g2] for sample s
            gcps = pss.tile([128, SPC * 4], mybir.dt.float32, tag="pa")
            # col layout: s*4 + dc*2 + q
            for s in range(SPC):
                for dc in range(2):
                    nc.tensor.matmul(
                        gcps[:, s * 4 + dc * 2:s * 4 + dc * 2 + 2],
                        lhsT=xn[:, s, dc * 128:(dc + 1) * 128],
                        rhs=mw[:, s, :], start=True, stop=True)
            gc = consts.tile([128, SPC * 4], bf, tag="c_gc")
            nc.vector.tensor_copy(gc[:, :], gcps[:, :])
            # pre-broadcast gcat along i so AT's moving operand streams packed
            gcb = consts.tile([128, 4, SPC * T], bf, tag="c_gcb")
            for c in range(4):
                cc = (c % 2) * 2 + c // 2
                nc.vector.tensor_copy(
                    gcb[:, c, :],
                    mk_ap(gc[:, cc:cc + 1], [[4, SPC], [0, T]]))

            # ---- per-head A''/B for ALL samples (built lazily) ----
            ab_cache = {}

            def get_ab(h):
                if h in ab_cache:
                    return ab_cache[h]
                full = (h == 3)
                jn = T if full else WIN
                natA = SPC * T
                nbtA = SPC * jn
                btsb = small.tile([128, 4, nbtA], bf, tag=f"btsb{h}")
                xs = xt if full else xtw
                for mf in range(4):
                    bps = pss.tile([128, nbtA], mybir.dt.float32, tag="pa")
                    mfs = slice(mf * 128, (mf + 1) * 128)
                    for kd in range(2):
                        nc.tensor.matmul(
                            bps[:, :], lhsT=w0[:, h, 2 + kd, mfs],
                            rhs=xs[:, kd, :, :],
                            start=(kd == 0), stop=(kd == 1))
                    fused_copy(btsb[:, mf, :], bps[:, :], None, nbtA,
                               relu=False)
                absb = small.tile([128, 4, natA], bf, tag=f"absb{h}")
                for mf in range(4):
                    aps = pss.tile([128, natA], mybir.dt.float32, tag="pa")
                    mfs = slice(mf * 128, (mf + 1) * 128)
                    for kd in range(2):
                        nc.tensor.matmul(
                            aps[:, :], lhsT=w0[:, h, kd, mfs],
                            rhs=xt[:, kd, :, :],
                            start=(kd == 0), stop=False)
                    for c in range(4):
                        nc.tensor.matmul(
                            aps[:, :], lhsT=w0[:, h, 4 + c, mfs],
                            rhs=gcb[:, c, :], start=False, stop=(c == 3))
                    fused_copy(absb[:, mf, :], aps[:, :],
                               bi[:, h * 4 + mf:h * 4 + mf + 1], natA,
                               relu=False)
                ab_cache[h] = (absb, btsb)
                return ab_cache[h]

            # head 3 in 4 groups of 2 samples; heads 0-2 one group of 8.
            # cg = 2048 columns uniformly.
            tasks = [(3, 0), (0, 0), (3, 1), (1, 0), (3, 2), (2, 0), (3, 3)]
            for ti, (h, g) in enumerate(tasks):
                    full = (h == 3)
                    SGh = 2 if full else SPC
                    s0 = g * SGh
                    cg = SGh * (CF if full else CW)  # group pair-columns
                    jn = T if full else WIN          # j per sample
                    absb, btsb = get_ab(h)

                    # ---- h0 = relu(A[:,s,i] + B[:,s,j]) ----
                    iN = T if full else WIN
                    h0sb = h0p.tile([128, 4, cg], bf, tag="h0")
                    sg2 = SGh // 2
                    cg2 = cg // 2
                    for hf_ in range(2):
                        sh = s0 + hf_ * sg2
                        for mf in range(4):
                            a_sl = absb[:, mf, sh * T:]
                            in0 = mk_ap(a_sl, [[T, sg2], [1, iN], [0, jn]])
                            b_sl = btsb[:, mf, sh * jn:]
                            in1 = mk_ap(b_sl, [[jn, sg2], [0, iN], [1, jn]])
                            out = h0sb[:, mf, hf_ * cg2:(hf_ + 1) * cg2] \
                                .rearrange("p (s i j) -> p s i j",
                                           s=sg2, i=iN)
                            eng_cost["dve"] += (151 + cg2) / 0.96
                            nc.vector.tensor_tensor(out=out, in0=in0,
                                                    in1=in1, op=ALU.add)
                        # in-place relu on DVE (single-src bf16 -> 4x mode)
                        eng_cost["dve"] += (58 + cg2) / 0.96
                        nc.vector.tensor_scalar_max(
                            h0sb[:, :, hf_ * cg2:(hf_ + 1) * cg2],
                            h0sb[:, :, hf_ * cg2:(hf_ + 1) * cg2], 0.0)

                    # ---- L1: 512 -> 256 ----
                    h1sb = actp.tile([128, 2, cg], bf, tag="h1")
                    for mf in range(2):
                        for np_ in range(cg // 1024):
                            ps = psb.tile([128, 1024], mybir.dt.float32,
                                          tag="ps")
                            for hf in range(2):
                                c0 = np_ * 1024 + hf * 512
                                for kd in range(4):
                                    nc.tensor.matmul(
                                        ps[:, hf * 512:(hf + 1) * 512],
                                        lhsT=w1[:, h, kd, mf, :],
                                        rhs=h0sb[:, kd, c0:c0 + 512],
                                        start=(kd == 0), stop=(kd == 3))
                            fused_copy(
                                h1sb[:, mf, np_ * 1024:(np_ + 1) * 1024],
                                ps[:, :],
                                bi[:, 16 + h * 2 + mf:17 + h * 2 + mf], 1024)

                    # ---- L2: 256 -> 128 ----
                    h2sb = actp.tile([128, cg], bf, tag="h2")
                    for np_ in range(cg // 1024):
                        ps = psb.tile([128, 1024], mybir.dt.float32, tag="ps")
                        for hf in range(2):
                            c0 = np_ * 1024 + hf * 512
                            for kd in range(2):
                                nc.tensor.matmul(
                                    ps[:, hf * 512:(hf + 1) * 512],
                                    lhsT=w2[:, h, kd, :],
                                    rhs=h1sb[:, kd, c0:c0 + 512],
                                    start=(kd == 0), stop=(kd == 1))
                        fused_copy(h2sb[:, np_ * 1024:(np_ + 1) * 1024],
                                   ps[:, :], bi[:, 24 + h:25 + h], 1024)

                    # ---- L3: 128 -> 64 ----
                    h3sb = actp.tile([F3, cg], bf, tag="h3")
                    for np_ in range(cg // 1024):
                        ps = psb.tile([F3, 1024], mybir.dt.float32, tag="ps")
                        for hf in range(2):
                            c0 = np_ * 1024 + hf * 512
                            nc.tensor.matmul(
                                ps[:, hf * 512:(hf + 1) * 512],
                                lhsT=w3[:, h, :], rhs=h2sb[:, c0:c0 + 512],
                                start=True, stop=True)
                        fused_copy(h3sb[:, np_ * 1024:(np_ + 1) * 1024],
                                   ps[:, :], bi[0:F3, 28 + h:29 + h], 1024)

                    # ---- L4: 64 -> 4 + finals ----
                    for np_ in range(cg // 1024):
                        ps4 = psb.tile([F4, 1024], mybir.dt.float32, tag="ps")
                        for hf in range(2):
                            c0 = np_ * 1024 + hf * 512
                            nc.tensor.matmul(
                                ps4[:, hf * 512:(hf + 1) * 512],
                                lhsT=w4[:, h, :], rhs=h3sb[:, c0:c0 + 512],
                                start=True, stop=True)
                        if full:
                            # chunk np_ is exactly sample s0+np_'s plane
                            nc.scalar.activation(
                                out=plane_sb[:, s0 + np_, :], in_=ps4[:, :],
                                func=AF.Sigmoid, bias=bi[0:F4, 32:33],
                                scale=1.0)
                            nc.sync.dma_start(
                                out=plane_d[:, s0 + np_, :],
                                in_=plane_sb[:, s0 + np_, :])
                        else:
                            for sl in range(1024 // CW):
                                s = s0 + np_ * (1024 // CW) + sl
                                scr = small.tile([F4, CW],
                                                 mybir.dt.float32, tag="scr")
                                nc.vector.scalar_tensor_tensor(
                                    out=scr[:, :],
                                    in0=ps4[:, sl * CW:(sl + 1) * CW],
                                    scalar=1.0, in1=wv[:, s, :],
                                    op0=ALU.mult, op1=ALU.mult,
                                    accum_out=sums_sb[:, s * 3 + h:
                                                      s * 3 + h + 1])

            nc.sync.dma_start(out=sums_d[:], in_=sums_sb[:])

    nc.compile()
    return nc


def _host_prep(emb, num_planes, w0, b0, w1, b1, w2, b2, w3, b3, w4, b4):
    emb = np.asarray(emb, np.float32)
    npl = np.asarray(num_planes).astype(np.int64)
    n0 = npl[:, 0]
    n1 = npl[:, 1]
    assert n0.min() >= 1 and n1.min() >= 1 and n0.max() <= 16 and n1.max() <= 16

    idx = np.arange(T)
    m1 = idx[None, :] < n0[:, None]
    in2 = (idx[None, :] >= n0[:, None]) & (idx[None, :] < (n0 + n1)[:, None])
    mw1 = (m1 / n0[:, None]).astype(np.float32)
    mw2 = (in2 / n1[:, None]).astype(np.float32)

    # xt: [128, kd, s, i] = emb[s, i, kd*128+p]
    embT = emb.transpose(2, 0, 1)                       # (D, B, T)
    xt = np.ascontiguousarray(
        embT.reshape(2, 128, B, T).transpose(1, 0, 2, 3)).astype(BF16)
    # window-gathered columns j = n0[s] + jj
    xtw_f = np.zeros((2, 128, B, WIN), np.float32)
    embT_r = embT.reshape(2, 128, B, T)
    for b in range(B):
        xtw_f[:, :, b, :] = embT_r[:, :, b, n0[b]:n0[b] + WIN]
    xtw = np.ascontiguousarray(xtw_f.transpose(1, 0, 2, 3)).astype(BF16)

    xnat = np.ascontiguousarray(emb.transpose(1, 0, 2)).astype(BF16)  # (T,B,D)
    mwt = np.ascontiguousarray(
        np.stack([mw1, mw2], axis=-1).transpose(1, 0, 2)).astype(BF16)

    w0s = np.ascontiguousarray(
        np.asarray(w0, np.float32).reshape(H, 8, 128, F0).transpose(0, 2, 1, 3)
    ).astype(BF16)
    w1s = np.ascontiguousarray(
        np.asarray(w1, np.float32).reshape(H, 4, 128, 2, 128)
        .transpose(0, 2, 1, 3, 4)).astype(BF16)
    w2s = np.ascontiguousarray(
        np.asarray(w2, np.float32).reshape(H, 2, 128, 128).transpose(2, 0, 1, 3)
    ).astype(BF16)
    w3s = np.ascontiguousarray(
        np.asarray(w3, np.float32).transpose(1, 0, 2)).astype(BF16)
    w4s = np.ascontiguousarray(
        np.asarray(w4, np.float32).transpose(1, 0, 2)).astype(BF16)
    b0row = np.ascontiguousarray(
        np.asarray(b0, np.float32).reshape(1, H, 4, 128)).astype(BF16)

    biases = np.zeros((128, 33), np.float32)
    biases[:, 0:16] = np.asarray(b0, np.float32).reshape(H, 4, 128) \
        .transpose(2, 0, 1).reshape(128, 16)
    biases[:, 16:24] = np.asarray(b1, np.float32).reshape(H, 2, 128) \
        .transpose(2, 0, 1).reshape(128, 8)
    biases[:, 24:28] = np.asarray(b2, np.float32).T
    biases[0:F3, 28:32] = np.asarray(b3, np.float32).T
    biases[0:F4, 32] = np.asarray(b4, np.float32)[3]

    cw = np.arange(CW)
    iw, jw = cw // WIN, cw % WIN
    pf = (n0 * n1).astype(np.float32)
    wvw = ((iw[None, :] < n0[:, None]) & (jw[None, :] < n1[:, None])) \
        / pf[:, None]
    wvw = np.broadcast_to(wvw[:, None, :].astype(np.float32), (B, F4, CW))

    in_maps = []
    for c in range(NCORES):
        sl = slice(c * SPC, (c + 1) * SPC)
        in_maps.append({
            "xt": np.ascontiguousarray(xt[:, :, sl, :]),
            "xtw": np.ascontiguousarray(xtw[:, :, sl, :]),
            "xnat": np.ascontiguousarray(xnat[:, sl, :]),
            "mwt": np.ascontiguousarray(mwt[:, sl, :]),
            "w0s": w0s, "w1s": w1s, "w2s": w2s, "w3s": w3s, "w4s": w4s,
            "b0row": b0row, "biases": biases,
            "wv": np.ascontiguousarray(wvw[sl].transpose(1, 0, 2)),
        })
    meta = dict(m1=m1, in2=in2, b4=np.asarray(b4, np.float32))
    return in_maps, meta


def _host_post(results, meta):
    b4 = meta["b4"]
    cam = np.zeros(B, np.float32)
    rot = np.zeros((B, 4), np.float32)
    trn = np.zeros((B, 3), np.float32)
    plane = np.zeros((B, T, T), np.float32)
    for c in range(NCORES):
        pl = results[c]["plane"]      # (4, SPC, CF)
        sm = results[c]["sums"]       # (4, SPC*3)
        for s in range(SPC):
            b = c * SPC + s
            plane[b] = pl[0, s].reshape(T, T)
            cam[b] = sm[0, s * 3 + 0] + b4[0, 0]
            rot[b] = sm[:, s * 3 + 1] + b4[1]
            trn[b] = sm[0:3, s * 3 + 2] + b4[2, :3]
    cam = (1.0 / (1.0 + np.exp(-cam.astype(np.float64)))).astype(np.float32)
    valid = meta["m1"][:, :, None] & meta["in2"][:, None, :]
    return cam, rot, trn, plane, valid


def kernel(emb, num_planes, w0, b0, w1, b1, w2, b2, w3, b3, w4, b4,
           _trace=False):
    global LAST_RESULTS
    from concourse.bass_utils import run_bass_kernel_spmd

    if "prog" not in _PROG_CACHE:
        _PROG_CACHE["prog"] = _build_program()
    nc = _PROG_CACHE["prog"]

    in_maps, meta = _host_prep(emb, num_planes, w0, b0, w1, b1,
                               w2, b2, w3, b3, w4, b4)
    res = run_bass_kernel_spmd(nc, in_maps, list(range(NCORES)),
                               trace=_trace)
    LAST_RESULTS = res
    return _host_post(res.results, meta)


# revision 31
# speedup vs baseline: 1.4243x; 1.0153x over previous
# Trainium2 Bass kernel for PlaneFormer-style pairwise-MLP head model.
#
# Data parallel over batch B=64 -> 8 samples per NeuronCore.  Per sample the
# reference computes cat[i,j] = [emb_i, emb_j, g1, g2] (T,T,4D) followed by 4
# stacked MLP heads (1024->512->256->128->64->4) and masked reductions.
#
# Device pipeline (v2), everything feature-major (features on partitions):
#   gcat  = [g1;g2] chunks via tiny transposed matmuls (lhsT = X natural)
#   A''   = X@w0i + gcat@w0g + b0   (PSUM accumulation; gcat enters as a
#           stride-0-broadcast moving operand pre-materialized via DVE,
#           b0 folded into the PSUM->SBUF copy bias)
#   B     = X@w0j  (head 3: all 32 j; heads 0-2: host-pregathered 16-col
#           window  j in [n0, n0+16) which provably contains all valid pairs)
#   h0    = relu(A''[:,i] + B[:,j])  -- DVE broadcast-AP add (bf16) plus
#           in-place DVE relu (4x mode), split into column halves so the
#           following L1 matmuls start at half-time
#   L1-L3 matmul chain with relu+bias fused into PSUM->SBUF copies
#           (balanced between ACT and DVE by an est-cost counter)
#   L4 head 3 -> sigmoid (ACT) -> plane logits;  heads 0-2 -> DVE
#           scalar_tensor_tensor with the exact validity/pf weights.
# Host only reshapes inputs, builds the tiny mask tensors from num_planes,
# and applies b4 / final sigmoid on the (64,)-sized reduced outputs.

import numpy as np
import ml_dtypes

B, T, D = 64, 32, 256
H = 4
NCORES = 8
SPC = B // NCORES          # samples per core
WIN = 16
CW = WIN * WIN             # 256 window pair-columns (heads 0-2)
CF = T * T                 # 1024 full pair-columns (head 3)
F0, F1, F2, F3, F4 = 512, 256, 128, 64, 4
SG = 4                     # samples per processing group
NG = SPC // SG

BF16 = ml_dtypes.bfloat16

_PROG_CACHE = {}
LAST_RESULTS = None


def _build_program():
    import concourse.bass as bass
    import concourse.tile as tile
    from concourse import bacc, mybir
    from contextlib import ExitStack

    f32 = mybir.dt.float32
    bf = mybir.dt.bfloat16
    AF = mybir.ActivationFunctionType
    ALU = mybir.AluOpType

    nc = bacc.Bacc("TRN2", target_bir_lowering=False, debug=False,
                   num_devices=NCORES)

    xt_d = nc.dram_tensor("xt", [128, 2, SPC, T], bf, kind="ExternalInput").ap()
    xtw_d = nc.dram_tensor("xtw", [128, 2, SPC, WIN], bf, kind="ExternalInput").ap()
    xn_d = nc.dram_tensor("xnat", [T, SPC, D], bf, kind="ExternalInput").ap()
    mw_d = nc.dram_tensor("mwt", [T, SPC, 2], bf, kind="ExternalInput").ap()
    w0_d = nc.dram_tensor("w0s", [H, 128, 8, F0], bf, kind="ExternalInput").ap()
    w1_d = nc.dram_tensor("w1s", [H, 128, 4, 2, 128], bf, kind="ExternalInput").ap()
    w2_d = nc.dram_tensor("w2s", [128, H, 2, 128], bf, kind="ExternalInput").ap()
    w3_d = nc.dram_tensor("w3s", [128, H, F3], bf, kind="ExternalInput").ap()
    w4_d = nc.dram_tensor("w4s", [F3, H, F4], bf, kind="ExternalInput").ap()
    b0r_d = nc.dram_tensor("b0row", [1, H, 4, 128], bf, kind="ExternalInput").ap()
    bias_d = nc.dram_tensor("biases", [128, 33], f32, kind="ExternalInput").ap()
    wv_d = nc.dram_tensor("wv", [F4, SPC, CW], f32, kind="ExternalInput").ap()

    plane_d = nc.dram_tensor("plane", [F4, SPC, CF], f32, kind="ExternalOutput").ap()
    sums_d = nc.dram_tensor("sums", [F4, SPC * 3], f32, kind="ExternalOutput").ap()

    with tile.TileContext(nc) as tc:
        with ExitStack() as ctx:
            consts = ctx.enter_context(tc.tile_pool(name="consts", bufs=1))
            small = ctx.enter_context(tc.tile_pool(name="small", bufs=3))
            h0p = ctx.enter_context(tc.tile_pool(name="h0p", bufs=2))
            actp = ctx.enter_context(tc.tile_pool(name="acts", bufs=2))
            outp = ctx.enter_context(tc.tile_pool(name="outs", bufs=1))
            psb = ctx.enter_context(tc.tile_pool(name="psb", bufs=3, space="PSUM"))
            pss = ctx.enter_context(tc.tile_pool(name="pss", bufs=2, space="PSUM"))

            def load_const(ap_d, shape, dt, tag, eng=None):
                t = consts.tile(shape, dt, tag=tag)
                (eng or nc.sync).dma_start(out=t[:], in_=ap_d[:])
                return t

            xn = load_const(xn_d, [T, SPC, D], bf, "c_xn", nc.gpsimd)
            mw = load_const(mw_d, [T, SPC, 2], bf, "c_mw", nc.gpsimd)
            xt = load_const(xt_d, [128, 2, SPC, T], bf, "c_xt", nc.gpsimd)
            xtw = load_const(xtw_d, [128, 2, SPC, WIN], bf, "c_xtw", nc.gpsimd)
            bi = load_const(bias_d, [128, 33], f32, "c_bi", nc.gpsimd)
            # per-head weight DMAs, head 3 first so its pipeline starts early
            w0 = consts.tile([128, H, 8, F0], bf, tag="c_w0")
            w1 = consts.tile([128, H, 4, 2, 128], bf, tag="c_w1")
            w2 = consts.tile([128, H, 2, 128], bf, tag="c_w2")
            w3 = consts.tile([128, H, F3], bf, tag="c_w3")
            w4 = consts.tile([F3, H, F4], bf, tag="c_w4")
            for hh in (3, 0, 1, 2):
                nc.sync.dma_start(out=w0[:, hh, :, :], in_=w0_d[hh])
                nc.sync.dma_start(out=w1[:, hh, :, :, :], in_=w1_d[hh])
            nc.sync.dma_start(out=w2[:], in_=w2_d[:])
            nc.sync.dma_start(out=w3[:], in_=w3_d[:])
            nc.sync.dma_start(out=w4[:], in_=w4_d[:])
            wv = load_const(wv_d, [F4, SPC, CW], f32, "c_wv", nc.gpsimd)

            plane_sb = outp.tile([F4, SPC, CF], f32)
            sums_sb = outp.tile([F4, SPC * 3], f32)

            eng_cost = {"act": 0.0, "dve": 0.0}

            def fused_copy(out_ap, in_ap, bias_ap, fd, relu=True):
                c_act = (172 + fd) / 1.2
                c_dve = (120 + fd) / 0.96
                if eng_cost["act"] + c_act <= eng_cost["dve"] + c_dve:
                    eng_cost["act"] += c_act
                    nc.scalar.activation(
                        out=out_ap, in_=in_ap,
                        func=(AF.Relu if relu else
                              (AF.Copy if bias_ap is None else AF.Identity)),
                        bias=(bias_ap if bias_ap is not None else 0.0),
                        scale=1.0)
                else:
                    eng_cost["dve"] += c_dve
                    if relu:
                        nc.vector.tensor_scalar(
                            out=out_ap, in0=in_ap,
                            scalar1=bias_ap if bias_ap is not None else 0.0,
                            scalar2=0.0, op0=ALU.add, op1=ALU.max)
                    elif bias_ap is None:
                        nc.vector.tensor_copy(out_ap, in_ap)
                    else:
                        nc.vector.tensor_scalar_add(out_ap, in_ap, bias_ap)

            def mk_ap(base_ap, dims):
                return bass.AP(tensor=base_ap.tensor, offset=base_ap.offset,
                               ap=[base_ap.ap[0]] + dims)

            # ---- gcat: per sample, 4 chunks (g1 lo/hi, g2 lo/hi) ----------
            # gcps[:, s*4+c] = chunk c of [g1;g2] for sample s
            gcps = pss.tile([128, SPC * 4], mybir.dt.float32, tag="pa")
            # col layout: s*4 + dc*2 + q
            for s in range(SPC):
                for dc in range(2):
                    nc.tensor.matmul(
                        gcps[:, s * 4 + dc * 2:s * 4 + dc * 2 + 2],
                        lhsT=xn[:, s, dc * 128:(dc + 1) * 128],
                        rhs=mw[:, s, :], start=True, stop=True)
            gc = consts.tile([128, SPC * 4], bf, tag="c_gc")
            eng_cost["dve"] += 2500.0   # gc + gcb prologue copies
            nc.vector.tensor_copy(gc[:, :], gcps[:, :])
            # pre-broadcast gcat along i so AT's moving operand streams packed
            gcb = consts.tile([128, 4, SPC * T], bf, tag="c_gcb")
            for c in range(4):
                cc = (c % 2) * 2 + c // 2
                nc.vector.tensor_copy(
                    gcb[:, c, :],
                    mk_ap(gc[:, cc:cc + 1], [[4, SPC], [0, T]]))

            # ---- per-head A''/B for ALL samples (built lazily) ----
            ab_cache = {}

            def get_ab(h):
                if h in ab_cache:
                    return ab_cache[h]
                full = (h == 3)
                jn = T if full else WIN
                natA = SPC * T
                nbtA = SPC * jn
                btsb = small.tile([128, 4, nbtA], bf, tag=f"btsb{h}")
                xs = xt if full else xtw
                for mf in range(4):
                    bps = pss.tile([128, nbtA], mybir.dt.float32, tag="pa")
                    mfs = slice(mf * 128, (mf + 1) * 128)
                    for kd in range(2):
                        nc.tensor.matmul(
                            bps[:, :], lhsT=w0[:, h, 2 + kd, mfs],
                            rhs=xs[:, kd, :, :],
                            start=(kd == 0), stop=(kd == 1))
                    fused_copy(btsb[:, mf, :], bps[:, :], None, nbtA,
                               relu=False)
                absb = small.tile([128, 4, natA], bf, tag=f"absb{h}")
                for mf in range(4):
                    aps = pss.tile([128, natA], mybir.dt.float32, tag="pa")
                    mfs = slice(mf * 128, (mf + 1) * 128)
                    for kd in range(2):
                        nc.tensor.matmul(
                            aps[:, :], lhsT=w0[:, h, kd, mfs],
                            rhs=xt[:, kd, :, :],
                            start=(kd == 0), stop=False)
                    for c in range(4):
                        nc.tensor.matmul(
                            aps[:, :], lhsT=w0[:, h, 4 + c, mfs],
                            rhs=gcb[:, c, :], start=False, stop=(c == 3))
                    fused_copy(absb[:, mf, :], aps[:, :],
                               bi[:, h * 4 + mf:h * 4 + mf + 1], natA,
                               relu=False)
                ab_cache[h] = (absb, btsb)
                return ab_cache[h]

            # head 3 in 4 groups of 2 samples; heads 0-2 one group of 8.
            # cg = 2048 columns uniformly.
            tasks = [(3, 0), (0, 0), (3, 1), (1, 0), (3, 2), (2, 0), (3, 3)]
            for ti, (h, g) in enumerate(tasks):
                    full = (h == 3)
                    SGh = 2 if full else SPC
                    s0 = g * SGh
                    cg = SGh * (CF if full else CW)  # group pair-columns
                    jn = T if full else WIN          # j per sample
                    absb, btsb = get_ab(h)

                    # ---- h0 = relu(A[:,s,i] + B[:,s,j]) ----
                    iN = T if full else WIN
                    h0sb = h0p.tile([128, 4, cg], bf, tag="h0")
                    sg2 = SGh // 2
                    cg2 = cg // 2
                    for hf_ in range(2):
                        sh = s0 + hf_ * sg2
                        for mf in range(4):
                            a_sl = absb[:, mf, sh * T:]
                            in0 = mk_ap(a_sl, [[T, sg2], [1, iN], [0, jn]])
                            b_sl = btsb[:, mf, sh * jn:]
                            in1 = mk_ap(b_sl, [[jn, sg2], [0, iN], [1, jn]])
                            out = h0sb[:, mf, hf_ * cg2:(hf_ + 1) * cg2] \
                                .rearrange("p (s i j) -> p s i j",
                                           s=sg2, i=iN)
                            eng_cost["dve"] += (151 + cg2) / 0.96
                            nc.vector.tensor_tensor(out=out, in0=in0,
                                                    in1=in1, op=ALU.add)
                        # in-place relu on DVE (single-src bf16 -> 4x mode)
                        eng_cost["dve"] += (58 + cg2) / 0.96
                        nc.vector.tensor_scalar_max(
                            h0sb[:, :, hf_ * cg2:(hf_ + 1) * cg2],
                            h0sb[:, :, hf_ * cg2:(hf_ + 1) * cg2], 0.0)

                    # ---- L1: 512 -> 256 ----
                    h1sb = actp.tile([128, 2, cg], bf, tag="h1")
                    for mf in range(2):
                        for np_ in range(cg // 1024):
                            ps = psb.tile([128, 1024], mybir.dt.float32,
                                          tag="ps")
                            for hf in range(2):
                                c0 = np_ * 1024 + hf * 512
                                for kd in range(4):
                                    nc.tensor.matmul(
                                        ps[:, hf * 512:(hf + 1) * 512],
                                        lhsT=w1[:, h, kd, mf, :],
                                        rhs=h0sb[:, kd, c0:c0 + 512],
                                        start=(kd == 0), stop=(kd == 3))
                            fused_copy(
                                h1sb[:, mf, np_ * 1024:(np_ + 1) * 1024],
                                ps[:, :],
                                bi[:, 16 + h * 2 + mf:17 + h * 2 + mf], 1024)

                    # ---- L2: 256 -> 128 ----
                    h2sb = actp.tile([128, cg], bf, tag="h2")
                    for np_ in range(cg // 1024):
                        ps = psb.tile([128, 1024], mybir.dt.float32, tag="ps")
                        for hf in range(2):
                            c0 = np_ * 1024 + hf * 512
                            for kd in range(2):
                                nc.tensor.matmul(
                                    ps[:, hf * 512:(hf + 1) * 512],
                                    lhsT=w2[:, h, kd, :],
                                    rhs=h1sb[:, kd, c0:c0 + 512],
                                    start=(kd == 0), stop=(kd == 1))
                        fused_copy(h2sb[:, np_ * 1024:(np_ + 1) * 1024],
                                   ps[:, :], bi[:, 24 + h:25 + h], 1024)

                    # ---- L3: 128 -> 64 ----
                    h3sb = actp.tile([F3, cg], bf, tag="h3")
                    for np_ in range(cg // 1024):
                        ps = psb.tile([F3, 1024], mybir.dt.float32, tag="ps")
                        for hf in range(2):
                            c0 = np_ * 1024 + hf * 512
                            nc.tensor.matmul(
                                ps[:, hf * 512:(hf + 1) * 512],
                                lhsT=w3[:, h, :], rhs=h2sb[:, c0:c0 + 512],
                                start=True, stop=True)
                        fused_copy(h3sb[:, np_ * 1024:(np_ + 1) * 1024],
                                   ps[:, :], bi[0:F3, 28 + h:29 + h], 1024)

                    # ---- L4: 64 -> 4 + finals ----
                    for np_ in range(cg // 1024):
                        ps4 = psb.tile([F4, 1024], mybir.dt.float32, tag="ps")
                        for hf in range(2):
                            c0 = np_ * 1024 + hf * 512
                            nc.tensor.matmul(
                                ps4[:, hf * 512:(hf + 1) * 512],
                                lhsT=w4[:, h, :], rhs=h3sb[:, c0:c0 + 512],
                                start=True, stop=True)
                        if full:
                            # chunk np_ is exactly sample s0+np_'s plane
                            nc.scalar.activation(
                                out=plane_sb[:, s0 + np_, :], in_=ps4[:, :],
                                func=AF.Sigmoid, bias=bi[0:F4, 32:33],
                                scale=1.0)
                            nc.sync.dma_start(
                                out=plane_d[:, s0 + np_, :],
                                in_=plane_sb[:, s0 + np_, :])
                        else:
                            for sl in range(1024 // CW):
                                s = s0 + np_ * (1024 // CW) + sl
                                scr = small.tile([F4, CW],
                                                 mybir.dt.float32, tag="scr")
                                eng_cost["dve"] += (120 + CW) / 0.96
                                nc.vector.scalar_tensor_tensor(
                                    out=scr[:, :],
                                    in0=ps4[:, sl * CW:(sl + 1) * CW],
                                    scalar=1.0, in1=wv[:, s, :],
                                    op0=ALU.mult, op1=ALU.mult,
                                    accum_out=sums_sb[:, s * 3 + h:
                                                      s * 3 + h + 1])

            nc.sync.dma_start(out=sums_d[:], in_=sums_sb[:])

    nc.compile()
    return nc


def _host_prep(emb, num_planes, w0, b0, w1, b1, w2, b2, w3, b3, w4, b4):
    emb = np.asarray(emb, np.float32)
    npl = np.asarray(num_planes).astype(np.int64)
    n0 = npl[:, 0]
    n1 = npl[:, 1]
    assert n0.min() >= 1 and n1.min() >= 1 and n0.max() <= 16 and n1.max() <= 16

    idx = np.arange(T)
    m1 = idx[None, :] < n0[:, None]
    in2 = (idx[None, :] >= n0[:, None]) & (idx[None, :] < (n0 + n1)[:, None])
    mw1 = (m1 / n0[:, None]).astype(np.float32)
    mw2 = (in2 / n1[:, None]).astype(np.float32)

    # xt: [128, kd, s, i] = emb[s, i, kd*128+p]
    embT = emb.transpose(2, 0, 1)                       # (D, B, T)
    xt = np.ascontiguousarray(
        embT.reshape(2, 128, B, T).transpose(1, 0, 2, 3)).astype(BF16)
    # window-gathered columns j = n0[s] + jj
    xtw_f = np.zeros((2, 128, B, WIN), np.float32)
    embT_r = embT.reshape(2, 128, B, T)
    for b in range(B):
        xtw_f[:, :, b, :] = embT_r[:, :, b, n0[b]:n0[b] + WIN]
    xtw = np.ascontiguousarray(xtw_f.transpose(1, 0, 2, 3)).astype(BF16)

    xnat = np.ascontiguousarray(emb.transpose(1, 0, 2)).astype(BF16)  # (T,B,D)
    mwt = np.ascontiguousarray(
        np.stack([mw1, mw2], axis=-1).transpose(1, 0, 2)).astype(BF16)

    w0s = np.ascontiguousarray(
        np.asarray(w0, np.float32).reshape(H, 8, 128, F0).transpose(0, 2, 1, 3)
    ).astype(BF16)
    w1s = np.ascontiguousarray(
        np.asarray(w1, np.float32).reshape(H, 4, 128, 2, 128)
        .transpose(0, 2, 1, 3, 4)).astype(BF16)
    w2s = np.ascontiguousarray(
        np.asarray(w2, np.float32).reshape(H, 2, 128, 128).transpose(2, 0, 1, 3)
    ).astype(BF16)
    w3s = np.ascontiguousarray(
        np.asarray(w3, np.float32).transpose(1, 0, 2)).astype(BF16)
    w4s = np.ascontiguousarray(
        np.asarray(w4, np.float32).transpose(1, 0, 2)).astype(BF16)
    b0row = np.ascontiguousarray(
        np.asarray(b0, np.float32).reshape(1, H, 4, 128)).astype(BF16)

    biases = np.zeros((128, 33), np.float32)
    biases[:, 0:16] = np.asarray(b0, np.float32).reshape(H, 4, 128) \
        .transpose(2, 0, 1).reshape(128, 16)
    biases[:, 16:24] = np.asarray(b1, np.float32).reshape(H, 2, 128) \
        .transpose(2, 0, 1).reshape(128, 8)
    biases[:, 24:28] = np.asarray(b2, np.float32).T
    biases[0:F3, 28:32] = np.asarray(b3, np.float32).T
    biases[0:F4, 32] = np.asarray(b4, np.float32)[3]

    cw = np.arange(CW)
    iw, jw = cw // WIN, cw % WIN
    pf = (n0 * n1).astype(np.float32)
    wvw = ((iw[None, :] < n0[:, None]) & (jw[None, :] < n1[:, None])) \
        / pf[:, None]
    wvw = np.broadcast_to(wvw[:, None, :].astype(np.float32), (B, F4, CW))

    in_maps = []
    for c in range(NCORES):
        sl = slice(c * SPC, (c + 1) * SPC)
        in_maps.append({
            "xt": np.ascontiguousarray(xt[:, :, sl, :]),
            "xtw": np.ascontiguousarray(xtw[:, :, sl, :]),
            "xnat": np.ascontiguousarray(xnat[:, sl, :]),
            "mwt": np.ascontiguousarray(mwt[:, sl, :]),
            "w0s": w0s, "w1s": w1s, "w2s": w2s, "w3s": w3s, "w4s": w4s,
            "b0row": b0row, "biases": biases,
            "wv": np.ascontiguousarray(wvw[sl].transpose(1, 0, 2)),
        })
    meta = dict(m1=m1, in2=in2, b4=np.asarray(b4, np.float32))
    return in_maps, meta


def _host_post(results, meta):
    b4 = meta["b4"]
    cam = np.zeros(B, np.float32)
    rot = np.zeros((B, 4), np.float32)
    trn = np.zeros((B, 3), np.float32)
    plane = np.zeros((B, T, T), np.float32)
    for c in range(NCORES):
        pl = results[c]["plane"]      # (4, SPC, CF)
        sm = results[c]["sums"]       # (4, SPC*3)
        for s in range(SPC):
            b = c * SPC + s
            plane[b] = pl[0, s].reshape(T, T)
            cam[b] = sm[0, s * 3 + 0] + b4[0, 0]
            rot[b] = sm[:, s * 3 + 1] + b4[1]
            trn[b] = sm[0:3, s * 3 + 2] + b4[2, :3]
    cam = (1.0 / (1.0 + np.exp(-cam.astype(np.float64)))).astype(np.float32)
    valid = meta["m1"][:, :, None] & meta["in2"][:, None, :]
    return cam, rot, trn, plane, valid


def kernel(emb, num_planes, w0, b0, w1, b1, w2, b2, w3, b3, w4, b4,
           _trace=False):
    global LAST_RESULTS
    from concourse.bass_utils import run_bass_kernel_spmd

    if "prog" not in _PROG_CACHE:
        _PROG_CACHE["prog"] = _build_program()
    nc = _PROG_CACHE["prog"]

    in_maps, meta = _host_prep(emb, num_planes, w0, b0, w1, b1,
                               w2, b2, w3, b3, w4, b4)
    res = run_bass_kernel_spmd(nc, in_maps, list(range(NCORES)),
                               trace=_trace)
    LAST_RESULTS = res
    return _host_post(res.results, meta)


# revision 32
# speedup vs baseline: 1.4264x; 1.0015x over previous
# Trainium2 Bass kernel for PlaneFormer-style pairwise-MLP head model.
#
# Data parallel over batch B=64 -> 8 samples per NeuronCore.  Per sample the
# reference computes cat[i,j] = [emb_i, emb_j, g1, g2] (T,T,4D) followed by 4
# stacked MLP heads (1024->512->256->128->64->4) and masked reductions.
#
# Device pipeline (v2), everything feature-major (features on partitions):
#   gcat  = [g1;g2] chunks via tiny transposed matmuls (lhsT = X natural)
#   A''   = X@w0i + gcat@w0g + b0   (PSUM accumulation; gcat enters as a
#           stride-0-broadcast moving operand pre-materialized via DVE,
#           b0 folded into the PSUM->SBUF copy bias)
#   B     = X@w0j  (head 3: all 32 j; heads 0-2: host-pregathered 16-col
#           window  j in [n0, n0+16) which provably contains all valid pairs)
#   h0    = relu(A''[:,i] + B[:,j])  -- DVE broadcast-AP add (bf16) plus
#           in-place DVE relu (4x mode), split into column halves so the
#           following L1 matmuls start at half-time
#   L1-L3 matmul chain with relu+bias fused into PSUM->SBUF copies
#           (balanced between ACT and DVE by an est-cost counter)
#   L4 head 3 -> sigmoid (ACT) -> plane logits;  heads 0-2 -> DVE
#           scalar_tensor_tensor with the exact validity/pf weights.
# Host only reshapes inputs, builds the tiny mask tensors from num_planes,
# and applies b4 / final sigmoid on the (64,)-sized reduced outputs.

import numpy as np
import ml_dtypes

B, T, D = 64, 32, 256
H = 4
NCORES = 8
SPC = B // NCORES          # samples per core
WIN = 16
CW = WIN * WIN             # 256 window pair-columns (heads 0-2)
CF = T * T                 # 1024 full pair-columns (head 3)
F0, F1, F2, F3, F4 = 512, 256, 128, 64, 4
SG = 4                     # samples per processing group
NG = SPC // SG

BF16 = ml_dtypes.bfloat16

_PROG_CACHE = {}
LAST_RESULTS = None


def _build_program():
    import concourse.bass as bass
    import concourse.tile as tile
    from concourse import bacc, mybir
    from contextlib import ExitStack

    f32 = mybir.dt.float32
    bf = mybir.dt.bfloat16
    AF = mybir.ActivationFunctionType
    ALU = mybir.AluOpType

    nc = bacc.Bacc("TRN2", target_bir_lowering=False, debug=False,
                   num_devices=NCORES)

    xt_d = nc.dram_tensor("xt", [128, 2, SPC, T], bf, kind="ExternalInput").ap()
    xtw_d = nc.dram_tensor("xtw", [128, 2, SPC, WIN], bf, kind="ExternalInput").ap()
    xn_d = nc.dram_tensor("xnat", [T, SPC, D], bf, kind="ExternalInput").ap()
    mw_d = nc.dram_tensor("mwt", [T, SPC, 2], bf, kind="ExternalInput").ap()
    w0_d = nc.dram_tensor("w0s", [H, 128, 8, F0], bf, kind="ExternalInput").ap()
    w1_d = nc.dram_tensor("w1s", [H, 128, 4, 2, 128], bf, kind="ExternalInput").ap()
    w2_d = nc.dram_tensor("w2s", [128, H, 2, 128], bf, kind="ExternalInput").ap()
    w3_d = nc.dram_tensor("w3s", [128, H, F3], bf, kind="ExternalInput").ap()
    w4_d = nc.dram_tensor("w4s", [F3, H, F4], bf, kind="ExternalInput").ap()
    b0r_d = nc.dram_tensor("b0row", [1, H, 4, 128], bf, kind="ExternalInput").ap()
    bias_d = nc.dram_tensor("biases", [128, 33], f32, kind="ExternalInput").ap()
    wv_d = nc.dram_tensor("wv", [F4, SPC, CW], f32, kind="ExternalInput").ap()

    plane_d = nc.dram_tensor("plane", [F4, SPC, CF], f32, kind="ExternalOutput").ap()
    sums_d = nc.dram_tensor("sums", [F4, SPC * 3], f32, kind="ExternalOutput").ap()

    with tile.TileContext(nc) as tc:
        with ExitStack() as ctx:
            consts = ctx.enter_context(tc.tile_pool(name="consts", bufs=1))
            small = ctx.enter_context(tc.tile_pool(name="small", bufs=3))
            h0p = ctx.enter_context(tc.tile_pool(name="h0p", bufs=2))
            actp = ctx.enter_context(tc.tile_pool(name="acts", bufs=2))
            outp = ctx.enter_context(tc.tile_pool(name="outs", bufs=1))
            psb = ctx.enter_context(tc.tile_pool(name="psb", bufs=3, space="PSUM"))
            pss = ctx.enter_context(tc.tile_pool(name="pss", bufs=2, space="PSUM"))

            def load_const(ap_d, shape, dt, tag, eng=None):
                t = consts.tile(shape, dt, tag=tag)
                (eng or nc.sync).dma_start(out=t[:], in_=ap_d[:])
                return t

            xn = load_const(xn_d, [T, SPC, D], bf, "c_xn")
            mw = load_const(mw_d, [T, SPC, 2], bf, "c_mw")
            xt = load_const(xt_d, [128, 2, SPC, T], bf, "c_xt", nc.gpsimd)
            xtw = load_const(xtw_d, [128, 2, SPC, WIN], bf, "c_xtw", nc.gpsimd)
            bi = load_const(bias_d, [128, 33], f32, "c_bi", nc.gpsimd)
            # per-head weight DMAs, head 3 first so its pipeline starts early
            w0 = consts.tile([128, H, 8, F0], bf, tag="c_w0")
            w1 = consts.tile([128, H, 4, 2, 128], bf, tag="c_w1")
            w2 = consts.tile([128, H, 2, 128], bf, tag="c_w2")
            w3 = consts.tile([128, H, F3], bf, tag="c_w3")
            w4 = consts.tile([F3, H, F4], bf, tag="c_w4")
            for hh in (3, 0, 1, 2):
                nc.sync.dma_start(out=w0[:, hh, :, :], in_=w0_d[hh])
                nc.sync.dma_start(out=w1[:, hh, :, :, :], in_=w1_d[hh])
            nc.sync.dma_start(out=w2[:], in_=w2_d[:])
            nc.sync.dma_start(out=w3[:], in_=w3_d[:])
            nc.sync.dma_start(out=w4[:], in_=w4_d[:])
            wv = load_const(wv_d, [F4, SPC, CW], f32, "c_wv", nc.gpsimd)

            plane_sb = outp.tile([F4, SPC, CF], f32)
            sums_sb = outp.tile([F4, SPC * 3], f32)

            eng_cost = {"act": 0.0, "dve": 0.0}

            def fused_copy(out_ap, in_ap, bias_ap, fd, relu=True):
                c_act = (172 + fd) / 1.2
                c_dve = (120 + fd) / 0.96
                if eng_cost["act"] + c_act <= eng_cost["dve"] + c_dve:
                    eng_cost["act"] += c_act
                    nc.scalar.activation(
                        out=out_ap, in_=in_ap,
                        func=(AF.Relu if relu else
                              (AF.Copy if bias_ap is None else AF.Identity)),
                        bias=(bias_ap if bias_ap is not None else 0.0),
                        scale=1.0)
                else:
                    eng_cost["dve"] += c_dve
                    if relu:
                        nc.vector.tensor_scalar(
                            out=out_ap, in0=in_ap,
                            scalar1=bias_ap if bias_ap is not None else 0.0,
                            scalar2=0.0, op0=ALU.add, op1=ALU.max)
                    elif bias_ap is None:
                        nc.vector.tensor_copy(out_ap, in_ap)
                    else:
                        nc.vector.tensor_scalar_add(out_ap, in_ap, bias_ap)

            def mk_ap(base_ap, dims):
                return bass.AP(tensor=base_ap.tensor, offset=base_ap.offset,
                               ap=[base_ap.ap[0]] + dims)

            # ---- gcat: per sample, 4 chunks (g1 lo/hi, g2 lo/hi) ----------
            # gcps[:, s*4+c] = chunk c of [g1;g2] for sample s
            gcps = pss.tile([128, SPC * 4], mybir.dt.float32, tag="pa")
            # col layout: s*4 + dc*2 + q
            for s in range(SPC):
                for dc in range(2):
                    nc.tensor.matmul(
                        gcps[:, s * 4 + dc * 2:s * 4 + dc * 2 + 2],
                        lhsT=xn[:, s, dc * 128:(dc + 1) * 128],
                        rhs=mw[:, s, :], start=True, stop=True)
            gc = consts.tile([128, SPC * 4], bf, tag="c_gc")
            eng_cost["dve"] += 2500.0   # gc + gcb prologue copies
            nc.vector.tensor_copy(gc[:, :], gcps[:, :])
            # pre-broadcast gcat along i so AT's moving operand streams packed
            gcb = consts.tile([128, 4, SPC * T], bf, tag="c_gcb")
            for c in range(4):
                cc = (c % 2) * 2 + c // 2
                nc.vector.tensor_copy(
                    gcb[:, c, :],
                    mk_ap(gc[:, cc:cc + 1], [[4, SPC], [0, T]]))

            # ---- per-head A''/B for ALL samples (built lazily) ----
            ab_cache = {}

            def get_ab(h):
                if h in ab_cache:
                    return ab_cache[h]
                full = (h == 3)
                jn = T if full else WIN
                natA = SPC * T
                nbtA = SPC * jn
                btsb = small.tile([128, 4, nbtA], bf, tag=f"btsb{h}")
                xs = xt if full else xtw
                for mf in range(4):
                    bps = pss.tile([128, nbtA], mybir.dt.float32, tag="pa")
                    mfs = slice(mf * 128, (mf + 1) * 128)
                    for kd in range(2):
                        nc.tensor.matmul(
                            bps[:, :], lhsT=w0[:, h, 2 + kd, mfs],
                            rhs=xs[:, kd, :, :],
                            start=(kd == 0), stop=(kd == 1))
                    fused_copy(btsb[:, mf, :], bps[:, :], None, nbtA,
                               relu=False)
                absb = small.tile([128, 4, natA], bf, tag=f"absb{h}")
                for mf in range(4):
                    aps = pss.tile([128, natA], mybir.dt.float32, tag="pa")
                    mfs = slice(mf * 128, (mf + 1) * 128)
                    for kd in range(2):
                        nc.tensor.matmul(
                            aps[:, :], lhsT=w0[:, h, kd, mfs],
                            rhs=xt[:, kd, :, :],
                            start=(kd == 0), stop=False)
                    for c in range(4):
                        nc.tensor.matmul(
                            aps[:, :], lhsT=w0[:, h, 4 + c, mfs],
                            rhs=gcb[:, c, :], start=False, stop=(c == 3))
                    fused_copy(absb[:, mf, :], aps[:, :],
                               bi[:, h * 4 + mf:h * 4 + mf + 1], natA,
                               relu=False)
                ab_cache[h] = (absb, btsb)
                return ab_cache[h]

            # head 3 in 4 groups of 2 samples; heads 0-2 one group of 8.
            # cg = 2048 columns uniformly.
            tasks = [(3, 0), (0, 0), (3, 1), (1, 0), (3, 2), (2, 0), (3, 3)]
            for ti, (h, g) in enumerate(tasks):
                    full = (h == 3)
                    SGh = 2 if full else SPC
                    s0 = g * SGh
                    cg = SGh * (CF if full else CW)  # group pair-columns
                    jn = T if full else WIN          # j per sample
                    absb, btsb = get_ab(h)

                    # ---- h0 = relu(A[:,s,i] + B[:,s,j]) ----
                    iN = T if full else WIN
                    h0sb = h0p.tile([128, 4, cg], bf, tag="h0")
                    sg2 = SGh // 2
                    cg2 = cg // 2
                    for hf_ in range(2):
                        sh = s0 + hf_ * sg2
                        for mf in range(4):
                            a_sl = absb[:, mf, sh * T:]
                            in0 = mk_ap(a_sl, [[T, sg2], [1, iN], [0, jn]])
                            b_sl = btsb[:, mf, sh * jn:]
                            in1 = mk_ap(b_sl, [[jn, sg2], [0, iN], [1, jn]])
                            out = h0sb[:, mf, hf_ * cg2:(hf_ + 1) * cg2] \
                                .rearrange("p (s i j) -> p s i j",
                                           s=sg2, i=iN)
                            eng_cost["dve"] += (151 + cg2) / 0.96
                            nc.vector.tensor_tensor(out=out, in0=in0,
                                                    in1=in1, op=ALU.add)
                        # in-place relu on DVE (single-src bf16 -> 4x mode)
                        eng_cost["dve"] += (58 + cg2) / 0.96
                        nc.vector.tensor_scalar_max(
                            h0sb[:, :, hf_ * cg2:(hf_ + 1) * cg2],
                            h0sb[:, :, hf_ * cg2:(hf_ + 1) * cg2], 0.0)

                    # ---- L1: 512 -> 256 ----
                    h1sb = actp.tile([128, 2, cg], bf, tag="h1")
                    for mf in range(2):
                        for np_ in range(cg // 1024):
                            ps = psb.tile([128, 1024], mybir.dt.float32,
                                          tag="ps")
                            for hf in range(2):
                                c0 = np_ * 1024 + hf * 512
                                for kd in range(4):
                                    nc.tensor.matmul(
                                        ps[:, hf * 512:(hf + 1) * 512],
                                        lhsT=w1[:, h, kd, mf, :],
                                        rhs=h0sb[:, kd, c0:c0 + 512],
                                        start=(kd == 0), stop=(kd == 3))
                            fused_copy(
                                h1sb[:, mf, np_ * 1024:(np_ + 1) * 1024],
                                ps[:, :],
                                bi[:, 16 + h * 2 + mf:17 + h * 2 + mf], 1024)

                    # ---- L2: 256 -> 128 ----
                    h2sb = actp.tile([128, cg], bf, tag="h2")
                    for np_ in range(cg // 1024):
                        ps = psb.tile([128, 1024], mybir.dt.float32, tag="ps")
                        for hf in range(2):
                            c0 = np_ * 1024 + hf * 512
                            for kd in range(2):
                                nc.tensor.matmul(
                                    ps[:, hf * 512:(hf + 1) * 512],
                                    lhsT=w2[:, h, kd, :],
                                    rhs=h1sb[:, kd, c0:c0 + 512],
                                    start=(kd == 0), stop=(kd == 1))
                        fused_copy(h2sb[:, np_ * 1024:(np_ + 1) * 1024],
                                   ps[:, :], bi[:, 24 + h:25 + h], 1024)

                    # ---- L3: 128 -> 64 ----
                    h3sb = actp.tile([F3, cg], bf, tag="h3")
                    for np_ in range(cg // 1024):
                        ps = psb.tile([F3, 1024], mybir.dt.float32, tag="ps")
                        for hf in range(2):
                            c0 = np_ * 1024 + hf * 512
                            nc.tensor.matmul(
                                ps[:, hf * 512:(hf + 1) * 512],
                                lhsT=w3[:, h, :], rhs=h2sb[:, c0:c0 + 512],
                                start=True, stop=True)
                        fused_copy(h3sb[:, np_ * 1024:(np_ + 1) * 1024],
                                   ps[:, :], bi[0:F3, 28 + h:29 + h], 1024)

                    # ---- L4: 64 -> 4 + finals ----
                    for np_ in range(cg // 1024):
                        ps4 = psb.tile([F4, 1024], mybir.dt.float32, tag="ps")
                        for hf in range(2):
                            c0 = np_ * 1024 + hf * 512
                            nc.tensor.matmul(
                                ps4[:, hf * 512:(hf + 1) * 512],
                                lhsT=w4[:, h, :], rhs=h3sb[:, c0:c0 + 512],
                                start=True, stop=True)
                        if full:
                            # chunk np_ is exactly sample s0+np_'s plane
                            nc.scalar.activation(
                                out=plane_sb[:, s0 + np_, :], in_=ps4[:, :],
                                func=AF.Sigmoid, bias=bi[0:F4, 32:33],
                                scale=1.0)
                            nc.sync.dma_start(
                                out=plane_d[:, s0 + np_, :],
                                in_=plane_sb[:, s0 + np_, :])
                        else:
                            for sl in range(1024 // CW):
                                s = s0 + np_ * (1024 // CW) + sl
                                scr = small.tile([F4, CW],
                                                 mybir.dt.float32, tag="scr")
                                eng_cost["dve"] += (120 + CW) / 0.96
                                nc.vector.scalar_tensor_tensor(
                                    out=scr[:, :],
                                    in0=ps4[:, sl * CW:(sl + 1) * CW],
                                    scalar=1.0, in1=wv[:, s, :],
                                    op0=ALU.mult, op1=ALU.mult,
                                    accum_out=sums_sb[:, s * 3 + h:
                                                      s * 3 + h + 1])

            nc.sync.dma_start(out=sums_d[:], in_=sums_sb[:])

    nc.compile()
    return nc


def _host_prep(emb, num_planes, w0, b0, w1, b1, w2, b2, w3, b3, w4, b4):
    emb = np.asarray(emb, np.float32)
    npl = np.asarray(num_planes).astype(np.int64)
    n0 = npl[:, 0]
    n1 = npl[:, 1]
    assert n0.min() >= 1 and n1.min() >= 1 and n0.max() <= 16 and n1.max() <= 16

    idx = np.arange(T)
    m1 = idx[None, :] < n0[:, None]
    in2 = (idx[None, :] >= n0[:, None]) & (idx[None, :] < (n0 + n1)[:, None])
    mw1 = (m1 / n0[:, None]).astype(np.float32)
    mw2 = (in2 / n1[:, None]).astype(np.float32)

    # xt: [128, kd, s, i] = emb[s, i, kd*128+p]
    embT = emb.transpose(2, 0, 1)                       # (D, B, T)
    xt = np.ascontiguousarray(
        embT.reshape(2, 128, B, T).transpose(1, 0, 2, 3)).astype(BF16)
    # window-gathered columns j = n0[s] + jj
    xtw_f = np.zeros((2, 128, B, WIN), np.float32)
    embT_r = embT.reshape(2, 128, B, T)
    for b in range(B):
        xtw_f[:, :, b, :] = embT_r[:, :, b, n0[b]:n0[b] + WIN]
    xtw = np.ascontiguousarray(xtw_f.transpose(1, 0, 2, 3)).astype(BF16)

    xnat = np.ascontiguousarray(emb.transpose(1, 0, 2)).astype(BF16)  # (T,B,D)
    mwt = np.ascontiguousarray(
        np.stack([mw1, mw2], axis=-1).transpose(1, 0, 2)).astype(BF16)

    w0s = np.ascontiguousarray(
        np.asarray(w0, np.float32).reshape(H, 8, 128, F0).transpose(0, 2, 1, 3)
    ).astype(BF16)
    w1s = np.ascontiguousarray(
        np.asarray(w1, np.float32).reshape(H, 4, 128, 2, 128)
        .transpose(0, 2, 1, 3, 4)).astype(BF16)
    w2s = np.ascontiguousarray(
        np.asarray(w2, np.float32).reshape(H, 2, 128, 128).transpose(2, 0, 1, 3)
    ).astype(BF16)
    w3s = np.ascontiguousarray(
        np.asarray(w3, np.float32).transpose(1, 0, 2)).astype(BF16)
    w4s = np.ascontiguousarray(
        np.asarray(w4, np.float32).transpose(1, 0, 2)).astype(BF16)
    b0row = np.ascontiguousarray(
        np.asarray(b0, np.float32).reshape(1, H, 4, 128)).astype(BF16)

    biases = np.zeros((128, 33), np.float32)
    biases[:, 0:16] = np.asarray(b0, np.float32).reshape(H, 4, 128) \
        .transpose(2, 0, 1).reshape(128, 16)
    biases[:, 16:24] = np.asarray(b1, np.float32).reshape(H, 2, 128) \
        .transpose(2, 0, 1).reshape(128, 8)
    biases[:, 24:28] = np.asarray(b2, np.float32).T
    biases[0:F3, 28:32] = np.asarray(b3, np.float32).T
    biases[0:F4, 32] = np.asarray(b4, np.float32)[3]

    cw = np.arange(CW)
    iw, jw = cw // WIN, cw % WIN
    pf = (n0 * n1).astype(np.float32)
    wvw = ((iw[None, :] < n0[:, None]) & (jw[None, :] < n1[:, None])) \
        / pf[:, None]
    wvw = np.broadcast_to(wvw[:, None, :].astype(np.float32), (B, F4, CW))

    in_maps = []
    for c in range(NCORES):
        sl = slice(c * SPC, (c + 1) * SPC)
        in_maps.append({
            "xt": np.ascontiguousarray(xt[:, :, sl, :]),
            "xtw": np.ascontiguousarray(xtw[:, :, sl, :]),
            "xnat": np.ascontiguousarray(xnat[:, sl, :]),
            "mwt": np.ascontiguousarray(mwt[:, sl, :]),
            "w0s": w0s, "w1s": w1s, "w2s": w2s, "w3s": w3s, "w4s": w4s,
            "b0row": b0row, "biases": biases,
            "wv": np.ascontiguousarray(wvw[sl].transpose(1, 0, 2)),
        })
    meta = dict(m1=m1, in2=in2, b4=np.asarray(b4, np.float32))
    return in_maps, meta


def _host_post(results, meta):
    b4 = meta["b4"]
    cam = np.zeros(B, np.float32)
    rot = np.zeros((B, 4), np.float32)
    trn = np.zeros((B, 3), np.float32)
    plane = np.zeros((B, T, T), np.float32)
    for c in range(NCORES):
        pl = results[c]["plane"]      # (4, SPC, CF)
        sm = results[c]["sums"]       # (4, SPC*3)
        for s in range(SPC):
            b = c * SPC + s
            plane[b] = pl[0, s].reshape(T, T)
            cam[b] = sm[0, s * 3 + 0] + b4[0, 0]
            rot[b] = sm[:, s * 3 + 1] + b4[1]
            trn[b] = sm[0:3, s * 3 + 2] + b4[2, :3]
    cam = (1.0 / (1.0 + np.exp(-cam.astype(np.float64)))).astype(np.float32)
    valid = meta["m1"][:, :, None] & meta["in2"][:, None, :]
    return cam, rot, trn, plane, valid


def kernel(emb, num_planes, w0, b0, w1, b1, w2, b2, w3, b3, w4, b4,
           _trace=False):
    global LAST_RESULTS
    from concourse.bass_utils import run_bass_kernel_spmd

    if "prog" not in _PROG_CACHE:
        _PROG_CACHE["prog"] = _build_program()
    nc = _PROG_CACHE["prog"]

    in_maps, meta = _host_prep(emb, num_planes, w0, b0, w1, b1,
                               w2, b2, w3, b3, w4, b4)
    res = run_bass_kernel_spmd(nc, in_maps, list(range(NCORES)),
                               trace=_trace)
    LAST_RESULTS = res
    return _host_post(res.results, meta)
